# revision 12
# baseline (speedup 1.0000x reference)
"""Trainium2 Bass kernel for CrossAttention with LoRA.

Data-parallel over batch (B=8 -> 8 NeuronCores, one batch element per core).
No collectives.

Fast path (the actual case: loralib-init B matrices and biases are all zero,
so LoRA/bias terms vanish): fully fused bf16 cross-attention with a T-chunked
pipeline -- see _build_fast. Key tricks:
  * all matmul operands bf16 (halves DMA/SBUF; PE cost unchanged; psum f32)
  * score matmuls for an even/odd head pair issued at PE row-tile positions
    (0,0)/(64,0) -- K=64 each, run concurrently in the PE array
  * attn@v uses a zero-padded M=128 stationary per head:
      even head [v(64) | 1 | 0...]: y -> psum rows 0:64, denom -> row 64
      odd  head [0... | 1 | v(64)]: denom -> row 63, y -> rows 64:128
    so every psum drain is partition-aligned (no SBUF->SBUF bounce)
  * attention is chunked over T (2 x 512): chunk 0's softmax-normalize and
    output projection overlap chunk 1's attention; q-projection of chunk 1
    fills PE gaps during chunk 0's attention (Tile's list scheduler pops
    ready work in emission order)
  * weights DMA'd in consumption order, packed contiguously per-tile on host

Slow path (any nonzero LoRA B / bias): the original f32r kernel, kept
verbatim for correctness insurance.
"""

import numpy as np

import concourse.bass as bass  # noqa: F401  (bass types via bacc)
import concourse.mybir as mybir
import concourse.tile as tile
from concourse import bacc
from concourse.bass_utils import run_bass_kernel_spmd

B, T, S, C, H, D, R = 8, 1024, 256, 1024, 16, 64, 16
SCALING = 1.0 / 16.0
P = 128
KC = C // P  # 8 k-tiles over the embedding dim
MT = T // P  # 8 tiles over T
NP = H // 2  # 8 head pairs
NCH = 2      # t-chunks (fast path) / psum chunks (slow path)
TCH = T // NCH
F32 = mybir.dt.float32
F32R = mybir.dt.float32r
BF16 = mybir.dt.bfloat16

_nc_cache: dict = {}


# --------------------------------------------------------------------------
# fast path
# --------------------------------------------------------------------------

def _build_fast():
    nc = bacc.Bacc("TRN2", target_bir_lowering=False, debug=False)

    # big tensors laid out contiguously per partition on host: [P, halves*4*X]
    xp = nc.declare_dram_parameter("xp", [P, KC * T], BF16, isOutput=False)
    fp = nc.declare_dram_parameter("fp", [P, KC * S], BF16, isOutput=False)
    wqp = nc.declare_dram_parameter("wqp", [P, KC * C], BF16, isOutput=False)
    wkp = nc.declare_dram_parameter("wkp", [P, KC * C], BF16, isOutput=False)
    wvp = nc.declare_dram_parameter("wvp", [P, KC * C], BF16, isOutput=False)
    wpp = nc.declare_dram_parameter("wpp", [P, KC * C], BF16, isOutput=False)
    mask0 = nc.declare_dram_parameter("mask0", [P, 2, P], BF16, isOutput=False)
    mask1 = nc.declare_dram_parameter("mask1", [P, 2, 2 * P], BF16, isOutput=False)
    onesb = nc.declare_dram_parameter("onesb", [P, NP], BF16, isOutput=False)
    eselb = nc.declare_dram_parameter("eselb", [H, C], BF16, isOutput=False)
    out = nc.declare_dram_parameter("out", [T, C], F32, isOutput=True)

    def cch(ch):
        return slice(ch * TCH, (ch + 1) * TCH)

    def c512(i):
        return slice(i * 512, (i + 1) * 512)

    with tile.TileContext(nc) as tc:
        with (
            tc.tile_pool(name="res", bufs=1) as res,
            tc.tile_pool(name="espool", bufs=4) as espool,
            tc.tile_pool(name="ostp", bufs=3) as ostp,
            tc.tile_pool(name="pp", bufs=3, space="PSUM") as pp,
            tc.tile_pool(name="psS", bufs=1, space="PSUM") as psS,
            tc.tile_pool(name="psy", bufs=3, space="PSUM") as psy,
        ):
            # ---- resident loads: 2 half-tensor DMAs per big tensor ---------
            # latency-critical (fT, Wfk) on the sync queue first; the rest
            # issued from the scalar queue (idle at start).
            def halves(name, X, view, engines):
                a = res.tile([P, 4, X], BF16, tag=f"{name}a", name=f"{name}a")
                b = res.tile([P, 4, X], BF16, tag=f"{name}b", name=f"{name}b")
                engines[0].dma_start(a[:], view[:, 0:4, :])
                engines[1].dma_start(b[:], view[:, 4:8, :])
                return a, b

            fp3 = fp.rearrange("p (k s) -> p k s", k=KC)
            xp3 = xp.rearrange("p (k t) -> p k t", k=KC)
            wq3 = wqp.rearrange("p (m c) -> p m c", m=KC)
            wk3 = wkp.rearrange("p (m c) -> p m c", m=KC)
            wv3 = wvp.rearrange("p (k c) -> p k c", k=KC)
            wp3 = wpp.rearrange("p (k c) -> p k c", k=KC)

            fT_sb = res.tile([P, KC, S], BF16, tag="fT", name="fT_sb")
            nc.sync.dma_start(fT_sb[:], fp3[:, :, :])
            wk_a, wk_b = halves("wk", C, wk3, (nc.sync, nc.sync))
            m0_sb = res.tile([P, 2, P], BF16, tag="m0", name="m0_sb")
            nc.sync.dma_start(m0_sb[:], mask0[:, :, :])
            m1_sb = res.tile([P, 2, 2 * P], BF16, tag="m1", name="m1_sb")
            nc.sync.dma_start(m1_sb[:], mask1[:, :, :])
            esel_sb = res.tile([H, C], BF16, tag="esel", name="esel_sb")
            nc.sync.dma_start(esel_sb[:], eselb[:, :])
            wv_a, wv_b = halves("wv", C, wv3, (nc.scalar, nc.scalar))
            x_a, x_b = halves("x", T, xp3, (nc.scalar, nc.scalar))
            wq_a, wq_b = halves("wq", C, wq3, (nc.scalar, nc.scalar))
            wp_a, wp_b = halves("wp", C, wp3, (nc.scalar, nc.scalar))

            def wfk(m):
                return (wk_a if m < 4 else wk_b)[:, m % 4, :]

            def wfv(k):
                return (wv_a if k < 4 else wv_b)[:, k % 4, :]

            def xT(k):
                return (x_a if k < 4 else x_b)[:, k % 4, :]

            def wq(m):
                return (wq_a if m < 4 else wq_b)[:, m % 4, :]

            def wp(k):
                return (wp_a if k < 4 else wp_b)[:, k % 4, :]

            fTs = [fT_sb[:, k, :] for k in range(KC)]

            # v_aug[s2]: [s(128), pair(8), parity(2), col(128)] bf16
            # even head (parity 0): cols 0:64 = v, col 64 = 1  -> y at psum
            #   rows 0:64, denominator at row 64
            # odd  head (parity 1): col 0 = 1, cols 64:128 = v -> denominator
            #   at psum row 0, y at rows 64:128
            # (all drains start at 32-aligned partitions, as walrus requires)
            v_aug = [
                res.tile([P, NP, 2, P], BF16, tag=f"vaug{s2}", name=f"vaug{s2}")
                for s2 in range(2)
            ]
            for s2 in range(2):
                nc.vector.memset(v_aug[s2][:], 0.0)
                nc.sync.dma_start(
                    v_aug[s2][:, :, 0:1, 64:65], onesb[:, :, None, None]
                )
                nc.sync.dma_start(
                    v_aug[s2][:, :, 1:2, 0:1], onesb[:, :, None, None]
                )

            # ---- k projection: kT[m] [128, S] ------------------------------
            kTs = [res.tile([P, S], BF16, tag=f"kT{m}", name=f"kT{m}") for m in range(KC)]
            for m in range(KC):
                ps = pp.tile([P, S], F32, tag="pp", name=f"k_ps{m}")
                for k in range(KC):
                    nc.tensor.matmul(
                        ps[:], wfk(m)[:, k * P:(k + 1) * P], fTs[k],
                        start=(k == 0), stop=(k == KC - 1),
                    )
                nc.vector.tensor_copy(kTs[m][:], ps[:])

            # ---- v projection into v_aug (lhsT held constant over cc) ------
            for s2 in range(2):
                pv = [
                    pp.tile([P, 4, 2, D], F32, tag="pp", name=f"v_ps{s2}_{cc}")
                    for cc in range(2)
                ]
                for k in range(KC):
                    for cc in range(2):
                        nc.tensor.matmul(
                            pv[cc][:], fTs[k][:, s2 * P:(s2 + 1) * P],
                            wfv(k)[:, c512(cc)],
                            start=(k == 0), stop=(k == KC - 1),
                        )
                for cc in range(2):
                    nc.vector.tensor_copy(
                        v_aug[s2][:, cc * 4:(cc + 1) * 4, 0:1, 0:D],
                        pv[cc][:, :, 0:1, :],
                    )
                    nc.vector.tensor_copy(
                        v_aug[s2][:, cc * 4:(cc + 1) * 4, 1:2, D:2 * D],
                        pv[cc][:, :, 1:2, :],
                    )

            qT = [
                [res.tile([P, TCH], BF16, tag=f"qT{ch}_{m}", name=f"qT{ch}_{m}")
                 for m in range(KC)]
                for ch in range(NCH)
            ]
            yT = [
                [res.tile([P, TCH], BF16, tag=f"yT{ch}_{p}", name=f"yT{ch}_{p}")
                 for p in range(NP)]
                for ch in range(NCH)
            ]
            rstage = [
                res.tile([P, NP * TCH], F32, tag=f"rstg{ch}", name=f"rstg{ch}")
                for ch in range(NCH)
            ]
            r_sb = [
                res.tile([H, TCH], F32, tag=f"rsb{ch}", name=f"rsb{ch}")
                for ch in range(NCH)
            ]
            recf = [
                res.tile([H, TCH], F32, tag=f"recf{ch}", name=f"recf{ch}")
                for ch in range(NCH)
            ]
            rec = [
                res.tile([H, TCH], BF16, tag=f"rec{ch}", name=f"rec{ch}")
                for ch in range(NCH)
            ]

            def qproj(ch):
                for m in range(KC):
                    ps = pp.tile([P, TCH], F32, tag="pp", name=f"q_ps{ch}_{m}")
                    for k in range(KC):
                        nc.tensor.matmul(
                            ps[:], wq(m)[:, k * P:(k + 1) * P], xT(k)[:, cch(ch)],
                            start=(k == 0), stop=(k == KC - 1),
                        )
                    nc.vector.tensor_copy(qT[ch][m][:], ps[:])

            def attention(ch):
                for p in range(NP):
                    es = [
                        espool.tile([P, 2, TCH], BF16, tag="es", name=f"es{ch}_{p}_{s2}")
                        for s2 in range(2)
                    ]
                    for s2 in range(2):
                        ps = psS.tile(
                            [P, 2 * TCH], F32, tag="psS", name=f"s_ps{ch}_{p}_{s2}"
                        )
                        nc.tensor.matmul(
                            ps[:, 0:TCH],
                            kTs[p][0:D, s2 * P:(s2 + 1) * P],
                            qT[ch][p][0:D, :],
                            start=True, stop=True,
                        )
                        nc.tensor.matmul(
                            ps[:, TCH:2 * TCH],
                            kTs[p][D:P, s2 * P:(s2 + 1) * P],
                            qT[ch][p][D:P, :],
                            start=True, stop=True,
                        )
                        nc.scalar.activation(
                            es[s2][:], ps[:],
                            mybir.ActivationFunctionType.Exp, scale=0.125,
                        )
                    if ch == 0:
                        nc.vector.tensor_mul(
                            es[0][:, :, 0:P], es[0][:, :, 0:P], m0_sb[:]
                        )
                        nc.vector.tensor_mul(
                            es[1][:, :, 0:2 * P], es[1][:, :, 0:2 * P], m1_sb[:]
                        )
                    psa = psy.tile([P, TCH], F32, tag="psy", name=f"ya{ch}_{p}")
                    psb = psy.tile([P, TCH], F32, tag="psy", name=f"yb{ch}_{p}")
                    for s2 in range(2):
                        nc.tensor.matmul(
                            psa[:], v_aug[s2][:, p:p + 1, 0:1, :],
                            es[s2][:, 0:1, :],
                            start=(s2 == 0), stop=(s2 == 1),
                        )
                    for s2 in range(2):
                        nc.tensor.matmul(
                            psb[:], v_aug[s2][:, p:p + 1, 1:2, :],
                            es[s2][:, 1:2, :],
                            start=(s2 == 0), stop=(s2 == 1),
                        )
                    pcols = slice(p * TCH, (p + 1) * TCH)
                    nc.vector.tensor_copy(yT[ch][p][0:D, :], psa[0:D, :])
                    nc.scalar.copy(rstage[ch][64:65, pcols], psa[64:65, :])
                    nc.vector.tensor_copy(yT[ch][p][D:P, :], psb[D:P, :])
                    nc.vector.tensor_copy(rstage[ch][0:1, pcols], psb[0:1, :])
                    # gather denom rows: r_sb rows 0:8 = odd heads, 8:16 = even
                    nc.sync.dma_start(
                        r_sb[ch][8 + p:9 + p, :], rstage[ch][64:65, pcols]
                    )
                    nc.sync.dma_start(
                        r_sb[ch][p:p + 1, :], rstage[ch][0:1, pcols]
                    )

            def norm(ch):
                nc.vector.reciprocal_approx_fast(recf[ch][:], r_sb[ch][:])
                nc.vector.tensor_copy(rec[ch][:], recf[ch][:])
                for p in range(NP):
                    rb = pp.tile([P, TCH], F32, tag="pp", name=f"rb{ch}_{p}")
                    nc.tensor.matmul(
                        rb[:], esel_sb[:, p * P:(p + 1) * P], rec[ch][:],
                        start=True, stop=True,
                    )
                    nc.vector.tensor_mul(yT[ch][p][:], yT[ch][p][:], rb[:])

            def outproj(ch):
                for mm in range(4):
                    m = ch * 4 + mm
                    po = [
                        pp.tile([P, 512], F32, tag="pp", name=f"o_ps{m}_{cc}")
                        for cc in range(2)
                    ]
                    for k in range(KC):
                        for cc in range(2):
                            nc.tensor.matmul(
                                po[cc][:], yT[ch][k][:, mm * P:(mm + 1) * P],
                                wp(k)[:, c512(cc)],
                                start=(k == 0), stop=(k == KC - 1),
                            )
                    for cc in range(2):
                        ost = ostp.tile([P, 512], F32, tag="ost", name=f"ost{m}_{cc}")
                        if cc == 0:
                            nc.scalar.copy(ost[:], po[cc][:])
                        else:
                            nc.vector.tensor_copy(ost[:], po[cc][:])
                        nc.sync.dma_start(out[m * P:(m + 1) * P, c512(cc)], ost[:])

            qproj(0)
            attention(0)
            qproj(1)
            attention(1)
            norm(0)
            outproj(0)
            norm(1)
            outproj(1)

    nc.finalize()
    return nc


def _host_prep_fast(x, feature, Wq, Wf, Wp):
    import ml_dtypes

    bf = ml_dtypes.bfloat16
    f32 = np.float32

    def pack_colblocks(W):
        # pack[p, m*C + k*128 + c] = W.T[k*128+p, m*128+c]
        WT = np.ascontiguousarray(np.asarray(W, f32).T)
        return np.ascontiguousarray(
            WT.reshape(KC, P, KC, P).transpose(1, 2, 0, 3).reshape(P, KC * C).astype(bf)
        )

    def pack_rows(M2d, X):
        # pack[p, k*X + t] = M2d[k*128+p, t]
        return np.ascontiguousarray(
            np.asarray(M2d, f32).reshape(KC, P, X).transpose(1, 0, 2)
            .reshape(P, KC * X).astype(bf)
        )

    i = np.arange(P)[:, None]
    j = np.arange(P)[None, :]
    m0 = (j >= i).astype(f32)
    j2 = np.arange(2 * P)[None, :]
    m1 = (j2 >= (P + i)).astype(f32)

    hsel = np.empty((H,), np.int64)
    hsel[:NP] = 2 * np.arange(NP) + 1
    hsel[NP:] = 2 * np.arange(NP)
    col = np.arange(C)[None, :]
    esel = (hsel[:, None] == col // D).astype(f32)

    shared = {
        "wqp": pack_colblocks(Wq),
        "wkp": pack_colblocks(Wf[:C]),
        "wvp": pack_rows(np.asarray(Wf[C:], f32).T, C),
        "wpp": pack_rows(np.asarray(Wp, f32).T, C),
        "mask0": np.ascontiguousarray(
            np.broadcast_to(m0[:, None, :], (P, 2, P)).astype(bf)
        ),
        "mask1": np.ascontiguousarray(
            np.broadcast_to(m1[:, None, :], (P, 2, 2 * P)).astype(bf)
        ),
        "onesb": np.ones((P, NP), bf),
        "eselb": np.ascontiguousarray(esel.astype(bf)),
    }
    in_maps = []
    for b in range(B):
        m = dict(shared)
        m["xp"] = pack_rows(np.asarray(x[b], f32).T, T)
        m["fp"] = pack_rows(np.asarray(feature[b], f32).T, S)
        in_maps.append(m)
    return in_maps


# --------------------------------------------------------------------------
# slow path (original f32r kernel; used only when LoRA B / bias are nonzero)
# --------------------------------------------------------------------------

def _build_slow(flags):
    has_lq, has_lf, has_lp, has_bq, has_bfk, has_bfv, has_bp = flags
    nc = bacc.Bacc("TRN2", target_bir_lowering=False, debug=False)

    xT = nc.declare_dram_parameter("xT", [C, T], F32R, isOutput=False)
    fT = nc.declare_dram_parameter("fT", [C, S], F32R, isOutput=False)
    WqT = nc.declare_dram_parameter("WqT", [C, C], F32R, isOutput=False)
    WfkT = nc.declare_dram_parameter("WfkT", [C, C], F32R, isOutput=False)
    WfvT = nc.declare_dram_parameter("WfvT", [C, C], F32R, isOutput=False)
    WpT = nc.declare_dram_parameter("WpT", [C, C], F32R, isOutput=False)
    mask = nc.declare_dram_parameter("mask", [P, 384], F32R, isOutput=False)
    vones = nc.declare_dram_parameter("vones", [P, H], F32R, isOutput=False)
    Esel = nc.declare_dram_parameter("Esel", [H, C], F32R, isOutput=False)
    if has_lq:
        AqT = nc.declare_dram_parameter("AqT", [C, R], F32R, isOutput=False)
        BqTs = nc.declare_dram_parameter("BqTs", [R, C], F32R, isOutput=False)
    if has_lf:
        AfT = nc.declare_dram_parameter("AfT", [C, R], F32R, isOutput=False)
        BfkTs = nc.declare_dram_parameter("BfkTs", [R, C], F32R, isOutput=False)
        BfvTs = nc.declare_dram_parameter("BfvTs", [R, C], F32R, isOutput=False)
    if has_lp:
        ApT = nc.declare_dram_parameter("ApT", [C, R], F32R, isOutput=False)
        BpTs = nc.declare_dram_parameter("BpTs", [R, C], F32R, isOutput=False)
    if has_bq:
        bq_pp = nc.declare_dram_parameter("bq_pp", [P, KC], F32, isOutput=False)
    if has_bfk:
        bfk_pp = nc.declare_dram_parameter("bfk_pp", [P, KC], F32, isOutput=False)
    if has_bfv:
        bfv_row = nc.declare_dram_parameter("bfv_row", [1, C], F32R, isOutput=False)
    if has_bp:
        bp_row = nc.declare_dram_parameter("bp_row", [1, C], F32R, isOutput=False)
    out = nc.declare_dram_parameter("out", [T, C], F32, isOutput=True)

    xT3 = xT.rearrange("(ko p) t -> ko p t", p=P)
    fT3 = fT.rearrange("(ko p) s -> ko p s", p=P)
    WqT3 = WqT.rearrange("(ko p) c -> p ko c", p=P)
    WfkT3 = WfkT.rearrange("(ko p) c -> p ko c", p=P)
    WfvT3 = WfvT.rearrange("(ko p) c -> ko p c", p=P)
    WpT3 = WpT.rearrange("(ko p) c -> ko p c", p=P)

    def c512(i):
        return slice(i * 512, (i + 1) * 512)

    with tile.TileContext(nc) as tc:
        with (
            tc.tile_pool(name="big", bufs=8) as big,
            tc.tile_pool(name="qpool", bufs=8) as qpool,
            tc.tile_pool(name="small", bufs=1) as small,
            tc.tile_pool(name="wcol", bufs=3) as wcol,
            tc.tile_pool(name="wrow", bufs=8) as wrow,
            tc.tile_pool(name="expp", bufs=6) as expp,
            tc.tile_pool(name="stg", bufs=3) as stg,
            tc.tile_pool(name="psA", bufs=4, space="PSUM") as psA,
            tc.tile_pool(name="psB", bufs=2, space="PSUM") as psB,
        ):
            xTs = [big.tile([P, T], F32R, tag="big", name=f"xT{k}") for k in range(KC)]
            for k in range(KC):
                nc.sync.dma_start(xTs[k][:], xT3[k])
            fTs = [small.tile([P, S], F32R, tag=f"fT{k}", name=f"fT{k}") for k in range(KC)]
            for k in range(KC):
                nc.sync.dma_start(fTs[k][:], fT3[k])
            mask_sb = small.tile([P, 384], F32R, tag="mask", name="mask_sb")
            nc.sync.dma_start(mask_sb[:], mask[:, :])
            esel_sb = small.tile([H, C], F32R, tag="esel", name="esel_sb")
            nc.sync.dma_start(esel_sb[:], Esel[:, :])
            if has_lq:
                aq_sb = small.tile([P, KC, R], F32R, tag="aq", name="aq_sb")
                nc.sync.dma_start(aq_sb[:], AqT.rearrange("(ko p) r -> p ko r", p=P))
                bqs_sb = small.tile([R, C], F32R, tag="bqs", name="bqs_sb")
                nc.sync.dma_start(bqs_sb[:], BqTs[:, :])
            if has_lf:
                af_sb = small.tile([P, KC, R], F32R, tag="af", name="af_sb")
                nc.sync.dma_start(af_sb[:], AfT.rearrange("(ko p) r -> p ko r", p=P))
                bfks_sb = small.tile([R, C], F32R, tag="bfks", name="bfks_sb")
                nc.sync.dma_start(bfks_sb[:], BfkTs[:, :])
                bfvs_sb = small.tile([R, C], F32R, tag="bfvs", name="bfvs_sb")
                nc.sync.dma_start(bfvs_sb[:], BfvTs[:, :])
            if has_lp:
                ap_sb = small.tile([P, KC, R], F32R, tag="ap", name="ap_sb")
                nc.sync.dma_start(ap_sb[:], ApT.rearrange("(ko p) r -> p ko r", p=P))
                bps_sb = small.tile([R, C], F32R, tag="bps", name="bps_sb")
                nc.sync.dma_start(bps_sb[:], BpTs[:, :])
            if has_bq:
                bq_sb = small.tile([P, KC], F32, tag="bq", name="bq_sb")
                nc.sync.dma_start(bq_sb[:], bq_pp[:, :])
            if has_bfk:
                bfk_sb = small.tile([P, KC], F32, tag="bfk", name="bfk_sb")
                nc.sync.dma_start(bfk_sb[:], bfk_pp[:, :])
            if has_bfv or has_bp:
                ones1 = small.tile([1, P], F32R, tag="ones1", name="ones1")
                nc.sync.dma_start(ones1[:], vones.rearrange("p h -> (p h)")[None, 0:P])
            if has_bfv:
                bfv_sb = small.tile([1, C], F32R, tag="bfv", name="bfv_sb")
                nc.sync.dma_start(bfv_sb[:], bfv_row[:, :])
            if has_bp:
                bp_sb = small.tile([1, C], F32R, tag="bp", name="bp_sb")
                nc.sync.dma_start(bp_sb[:], bp_row[:, :])

            if has_lq:
                ups = psB.tile([P, T], F32, tag="y", name="uq_ps")
                for ch in range(NCH):
                    for k in range(KC):
                        nc.tensor.matmul(
                            ups[:R, c512(ch)], aq_sb[:, k, :], xTs[k][:, c512(ch)],
                            start=(k == 0), stop=(k == KC - 1),
                        )
                uq_sb = small.tile([R, T], F32R, tag="uq", name="uq_sb")
                nc.scalar.copy(uq_sb[:], ups[:R, :])
            if has_lf:
                ufs = psB.tile([P, T], F32, tag="y", name="uf_ps")
                for k in range(KC):
                    nc.tensor.matmul(
                        ufs[:R, :S], af_sb[:, k, :], fTs[k][:],
                        start=(k == 0), stop=(k == KC - 1),
                    )
                uf_sb = small.tile([R, S], F32R, tag="uf", name="uf_sb")
                nc.scalar.copy(uf_sb[:], ufs[:R, :S])

            kTs = [small.tile([P, S], F32R, tag=f"kT{m}", name=f"kT{m}") for m in range(KC)]
            for m in range(KC):
                wk_m = wcol.tile([P, KC, P], F32R, tag="wcol", name=f"wk{m}")
                nc.sync.dma_start(wk_m[:], WfkT3[:, :, m * P:(m + 1) * P])
                ps = psA.tile([P, S], F32, tag="mm", name=f"k_ps{m}")
                for k in range(KC):
                    nc.tensor.matmul(
                        ps[:], wk_m[:, k, :], fTs[k][:],
                        start=(k == 0), stop=(k == KC - 1 and not has_lf),
                    )
                if has_lf:
                    nc.tensor.matmul(
                        ps[:], bfks_sb[:, m * P:(m + 1) * P], uf_sb[:],
                        start=False, stop=True,
                    )
                if has_bfk:
                    nc.scalar.activation(
                        kTs[m][:], ps[:], mybir.ActivationFunctionType.Identity,
                        bias=bfk_sb[:, m:m + 1], scale=1.0,
                    )
                else:
                    nc.vector.tensor_copy(kTs[m][:], ps[:])

            wfv = [wrow.tile([P, C], F32R, tag="wrow", name=f"wfv{k}") for k in range(KC)]
            for k in range(KC):
                nc.sync.dma_start(wfv[k][:], WfvT3[k])
            v_aug = [
                small.tile([P, H, D + 1], F32R, tag=f"vaug{s2}", name=f"vaug{s2}")
                for s2 in range(2)
            ]
            for s2 in range(2):
                nc.sync.dma_start(v_aug[s2][:, :, D], vones[:, :])
                for ch in range(NCH):
                    ps = psA.tile([P, 512], F32, tag="mm", name=f"v_ps{s2}_{ch}")
                    nmm = KC + (1 if has_lf else 0) + (1 if has_bfv else 0)
                    i = 0
                    for k in range(KC):
                        i += 1
                        nc.tensor.matmul(
                            ps[:], fTs[k][:, s2 * P:(s2 + 1) * P],
                            wfv[k][:, c512(ch)],
                            start=(i == 1), stop=(i == nmm),
                        )
                    if has_lf:
                        i += 1
                        nc.tensor.matmul(
                            ps[:], uf_sb[:, s2 * P:(s2 + 1) * P],
                            bfvs_sb[:, c512(ch)], start=False, stop=(i == nmm),
                        )
                    if has_bfv:
                        i += 1
                        nc.tensor.matmul(
                            ps[:], ones1[:], bfv_sb[:, c512(ch)],
                            start=False, stop=(i == nmm),
                        )
                    for hh in range(8):
                        h = ch * 8 + hh
                        nc.vector.tensor_copy(
                            v_aug[s2][:, h, 0:D], ps[:, hh * D:(hh + 1) * D]
                        )

            qTs = [qpool.tile([P, T], F32R, tag="qT", name=f"qT{m}") for m in range(MT)]
            for m in range(KC):
                wq_m = wcol.tile([P, KC, P], F32R, tag="wcol", name=f"wq{m}")
                nc.sync.dma_start(wq_m[:], WqT3[:, :, m * P:(m + 1) * P])
                for ch in range(NCH):
                    ps = psA.tile([P, 512], F32, tag="mm", name=f"q_ps{m}_{ch}")
                    for k in range(KC):
                        nc.tensor.matmul(
                            ps[:], wq_m[:, k, :], xTs[k][:, c512(ch)],
                            start=(k == 0), stop=(k == KC - 1 and not has_lq),
                        )
                    if has_lq:
                        nc.tensor.matmul(
                            ps[:], bqs_sb[:, m * P:(m + 1) * P], uq_sb[:, c512(ch)],
                            start=False, stop=True,
                        )
                    if has_bq:
                        nc.scalar.activation(
                            qTs[m][:, c512(ch)], ps[:],
                            mybir.ActivationFunctionType.Identity,
                            bias=bq_sb[:, m:m + 1], scale=1.0,
                        )
                    else:
                        nc.vector.tensor_copy(qTs[m][:, c512(ch)], ps[:])

            yTr = [big.tile([P, T], F32R, tag="big", name=f"yTr{p}") for p in range(KC)]
            r_sb = small.tile([H, T], F32R, tag="rsum", name="r_sb")
            for h in range(H):
                m, off = h // 2, (h % 2) * D
                kt_h = kTs[m][off:off + D, :]
                qt_h = qTs[m][off:off + D, :]
                es = [expp.tile([P, T], F32R, tag="exp", name=f"e{h}_{s2}") for s2 in range(2)]
                for s2 in range(2):
                    for ch in range(NCH):
                        ps = psA.tile([P, 512], F32, tag="mm", name=f"s_ps{h}_{s2}_{ch}")
                        nc.tensor.matmul(
                            ps[:], kt_h[:, s2 * P:(s2 + 1) * P], qt_h[:, c512(ch)],
                            start=True, stop=True,
                        )
                        nc.scalar.activation(
                            es[s2][:, c512(ch)], ps[:],
                            mybir.ActivationFunctionType.Exp, scale=0.125,
                        )
                nc.vector.tensor_mul(es[0][:, 0:P], es[0][:, 0:P], mask_sb[:, 0:P])
                nc.vector.tensor_mul(es[1][:, 0:S], es[1][:, 0:S], mask_sb[:, P:384])
                psy = psB.tile([P, T], F32, tag="y", name=f"y_ps{h}")
                for ch in range(NCH):
                    for s2 in range(2):
                        nc.tensor.matmul(
                            psy[:D + 1, c512(ch)], v_aug[s2][:, h, :],
                            es[s2][:, c512(ch)], start=(s2 == 0), stop=(s2 == 1),
                        )
                st = stg.tile([P, T], F32R, tag="hstage", name=f"st{h}")
                if off == 0:
                    nc.vector.tensor_copy(yTr[m][0:D, :], psy[0:D, :])
                    nc.vector.tensor_copy(st[D:D + 1, :], psy[D:D + 1, :])
                else:
                    nc.vector.tensor_copy(st[0:D + 1, :], psy[0:D + 1, :])
                    nc.sync.dma_start(yTr[m][off:off + D, :], st[0:D, :])
                nc.sync.dma_start(r_sb[h:h + 1, :], st[D:D + 1, :])

            recf = small.tile([H, T], F32, tag="recf", name="recf")
            nc.vector.reciprocal(recf[:], r_sb[:])
            rec = small.tile([H, T], F32R, tag="rec", name="rec")
            nc.vector.tensor_copy(rec[:], recf[:])
            for p in range(KC):
                rb = psB.tile([P, T], F32, tag="y", name=f"rb{p}")
                for ch in range(NCH):
                    nc.tensor.matmul(
                        rb[:, c512(ch)], esel_sb[:, p * P:(p + 1) * P],
                        rec[:, c512(ch)], start=True, stop=True,
                    )
                nc.vector.tensor_mul(yTr[p][:], yTr[p][:], rb[:])

            if has_lp:
                upsd = psB.tile([P, T], F32, tag="y", name="up_ps")
                for ch in range(NCH):
                    for k in range(KC):
                        nc.tensor.matmul(
                            upsd[:R, c512(ch)], ap_sb[:, k, :], yTr[k][:, c512(ch)],
                            start=(k == 0), stop=(k == KC - 1),
                        )
                up_sb = small.tile([R, T], F32R, tag="up", name="up_sb")
                nc.scalar.copy(up_sb[:], upsd[:R, :])
            wp = [wrow.tile([P, C], F32R, tag="wrow", name=f"wp{k}") for k in range(KC)]
            for k in range(KC):
                nc.sync.dma_start(wp[k][:], WpT3[k])
            for m in range(MT):
                for ch in range(NCH):
                    ps = psA.tile([P, 512], F32, tag="mm", name=f"o_ps{m}_{ch}")
                    nmm = KC + (1 if has_lp else 0) + (1 if has_bp else 0)
                    i = 0
                    for k in range(KC):
                        i += 1
                        nc.tensor.matmul(
                            ps[:], yTr[k][:, m * P:(m + 1) * P], wp[k][:, c512(ch)],
                            start=(i == 1), stop=(i == nmm),
                        )
                    if has_lp:
                        i += 1
                        nc.tensor.matmul(
                            ps[:], up_sb[:, m * P:(m + 1) * P], bps_sb[:, c512(ch)],
                            start=False, stop=(i == nmm),
                        )
                    if has_bp:
                        i += 1
                        nc.tensor.matmul(
                            ps[:], ones1[:], bp_sb[:, c512(ch)],
                            start=False, stop=(i == nmm),
                        )
                    ost = wcol.tile([P, 512], F32, tag="ostage", name=f"ost{m}_{ch}")
                    nc.scalar.copy(ost[:], ps[:])
                    nc.sync.dma_start(out[m * P:(m + 1) * P, c512(ch)], ost[:])

    nc.finalize()
    return nc


def _host_prep_slow(x, feature, Wq, bq, Aq, Bq, Wf, bf, Af, Bf, Wp, bp, Ap, Bp, flags):
    f32 = np.float32
    shared = {
        "WqT": np.ascontiguousarray(np.asarray(Wq, f32).T),
        "WfkT": np.ascontiguousarray(np.asarray(Wf[:C], f32).T),
        "WfvT": np.ascontiguousarray(np.asarray(Wf[C:], f32).T),
        "WpT": np.ascontiguousarray(np.asarray(Wp, f32).T),
    }
    i = np.arange(P)[:, None]
    j = np.arange(384)[None, :]
    m0 = (j[:, :P] >= i).astype(f32)
    m1 = ((j[:, P:384] - P) >= (P + i)).astype(f32)
    shared["mask"] = np.ascontiguousarray(np.concatenate([m0, m1], axis=1))
    shared["vones"] = np.ones((P, H), f32)
    hsel = np.arange(H)[:, None]
    col = np.arange(C)[None, :]
    shared["Esel"] = np.ascontiguousarray((hsel == col // D).astype(f32))
    has_lq, has_lf, has_lp, has_bq, has_bfk, has_bfv, has_bp = flags
    if has_lq:
        shared["AqT"] = np.ascontiguousarray(np.asarray(Aq, f32).T)
        shared["BqTs"] = np.ascontiguousarray(np.asarray(Bq, f32).T * SCALING)
    if has_lf:
        shared["AfT"] = np.ascontiguousarray(np.asarray(Af, f32).T)
        shared["BfkTs"] = np.ascontiguousarray(np.asarray(Bf[:C], f32).T * SCALING)
        shared["BfvTs"] = np.ascontiguousarray(np.asarray(Bf[C:], f32).T * SCALING)
    if has_lp:
        shared["ApT"] = np.ascontiguousarray(np.asarray(Ap, f32).T)
        shared["BpTs"] = np.ascontiguousarray(np.asarray(Bp, f32).T * SCALING)
    if has_bq:
        shared["bq_pp"] = np.ascontiguousarray(np.asarray(bq, f32).reshape(KC, P).T)
    if has_bfk:
        shared["bfk_pp"] = np.ascontiguousarray(np.asarray(bf[:C], f32).reshape(KC, P).T)
    if has_bfv:
        shared["bfv_row"] = np.ascontiguousarray(np.asarray(bf[C:], f32).reshape(1, C))
    if has_bp:
        shared["bp_row"] = np.ascontiguousarray(np.asarray(bp, f32).reshape(1, C))

    in_maps = []
    for b in range(B):
        m = dict(shared)
        m["xT"] = np.ascontiguousarray(np.asarray(x[b], f32).T)
        m["fT"] = np.ascontiguousarray(np.asarray(feature[b], f32).T)
        in_maps.append(m)
    return in_maps


# --------------------------------------------------------------------------
# dispatch
# --------------------------------------------------------------------------

def _run(inputs, trace=False, **spmd_kwargs):
    x, feature = inputs["x"], inputs["feature"]
    Wq, bq, Aq, Bq = inputs["Wq"], inputs["bq"], inputs["Aq"], inputs["Bq"]
    Wf, bf_, Af, Bf = inputs["Wf"], inputs["bf"], inputs["Af"], inputs["Bf"]
    Wp, bp, Ap, Bp = inputs["Wp"], inputs["bp"], inputs["Ap"], inputs["Bp"]
    flags = (
        bool(np.any(Bq)), bool(np.any(Bf)), bool(np.any(Bp)),
        bool(np.any(bq)), bool(np.any(bf_[:C])), bool(np.any(bf_[C:])),
        bool(np.any(bp)),
    )
    if any(flags):
        key = ("slow", flags)
        nc = _nc_cache.get(key)
        if nc is None:
            nc = _build_slow(flags)
            _nc_cache[key] = nc
        in_maps = _host_prep_slow(
            x, feature, Wq, bq, Aq, Bq, Wf, bf_, Af, Bf, Wp, bp, Ap, Bp, flags
        )
    else:
        key = "fast"
        nc = _nc_cache.get(key)
        if nc is None:
            nc = _build_fast()
            _nc_cache[key] = nc
        in_maps = _host_prep_fast(x, feature, Wq, Wf, Wp)
    res = run_bass_kernel_spmd(
        nc, in_maps, core_ids=list(range(B)), trace=trace, **spmd_kwargs
    )
    out = np.stack([res.results[b]["out"] for b in range(B)], axis=0)
    return out, res


def kernel(**inputs):
    out, _ = _run(inputs, trace=False)
    return out


# revision 13
# speedup vs baseline: 1.0609x; 1.0609x over previous
"""Trainium2 Bass kernel for CrossAttention with LoRA.

Data-parallel over batch (B=8 -> 8 NeuronCores, one batch element per core).
No collectives.

Fast path (the actual case: loralib-init B matrices and biases are all zero,
so LoRA/bias terms vanish): fully fused bf16 cross-attention with a T-chunked
pipeline -- see _build_fast. Key tricks:
  * all matmul operands bf16 (halves DMA/SBUF; PE cost unchanged; psum f32)
  * score matmuls for an even/odd head pair issued at PE row-tile positions
    (0,0)/(64,0) -- K=64 each, run concurrently in the PE array
  * attn@v uses a zero-padded M=128 stationary per head:
      even head [v(64) | 1 | 0...]: y -> psum rows 0:64, denom -> row 64
      odd  head [0... | 1 | v(64)]: denom -> row 63, y -> rows 64:128
    so every psum drain is partition-aligned (no SBUF->SBUF bounce)
  * attention is chunked over T (2 x 512): chunk 0's softmax-normalize and
    output projection overlap chunk 1's attention; q-projection of chunk 1
    fills PE gaps during chunk 0's attention (Tile's list scheduler pops
    ready work in emission order)
  * weights DMA'd in consumption order, packed contiguously per-tile on host

Slow path (any nonzero LoRA B / bias): the original f32r kernel, kept
verbatim for correctness insurance.
"""

import numpy as np

import concourse.bass as bass  # noqa: F401  (bass types via bacc)
import concourse.mybir as mybir
import concourse.tile as tile
from concourse import bacc
from concourse.bass_utils import run_bass_kernel_spmd

B, T, S, C, H, D, R = 8, 1024, 256, 1024, 16, 64, 16
SCALING = 1.0 / 16.0
P = 128
KC = C // P  # 8 k-tiles over the embedding dim
MT = T // P  # 8 tiles over T
NP = H // 2  # 8 head pairs
NCH = 2      # t-chunks (fast path) / psum chunks (slow path)
TCH = T // NCH
F32 = mybir.dt.float32
F32R = mybir.dt.float32r
BF16 = mybir.dt.bfloat16

_nc_cache: dict = {}


# --------------------------------------------------------------------------
# fast path
# --------------------------------------------------------------------------

def _build_fast():
    nc = bacc.Bacc("TRN2", target_bir_lowering=False, debug=False)

    # host-packed contiguous per-partition layouts:
    #   xp{ch}[p, k*TCH + t] = x.T[k*128+p, ch*TCH+t]
    #   fp[p, k*S + s]       = f.T[k*128+p, s]
    #   wqp/wkp[p, m*C + k*128 + c] = W.T[k*128+p, m*128+c]   (m-major)
    #   wvp/wpp[p, k*C + c]  = W.T[k*128+p, c]                (k-major)
    xp0 = nc.declare_dram_parameter("xp0", [P, KC * TCH], BF16, isOutput=False)
    xp1 = nc.declare_dram_parameter("xp1", [P, KC * TCH], BF16, isOutput=False)
    fp = nc.declare_dram_parameter("fp", [P, KC * S], BF16, isOutput=False)
    wqp = nc.declare_dram_parameter("wqp", [P, KC * C], BF16, isOutput=False)
    wkp = nc.declare_dram_parameter("wkp", [P, KC * C], BF16, isOutput=False)
    wvp = nc.declare_dram_parameter("wvp", [P, KC * C], BF16, isOutput=False)
    wpp = nc.declare_dram_parameter("wpp", [P, KC * C], BF16, isOutput=False)
    mask0 = nc.declare_dram_parameter("mask0", [P, 2, P], BF16, isOutput=False)
    mask1 = nc.declare_dram_parameter("mask1", [P, 2, 2 * P], BF16, isOutput=False)
    onesb = nc.declare_dram_parameter("onesb", [P, NP], BF16, isOutput=False)
    eselb = nc.declare_dram_parameter("eselb", [H, C], BF16, isOutput=False)
    out = nc.declare_dram_parameter("out", [T, C], F32, isOutput=True)

    HC = 4 * C

    def c512(i):
        return slice(i * 512, (i + 1) * 512)

    with tile.TileContext(nc) as tc:
        with (
            tc.tile_pool(name="res", bufs=1) as res,
            tc.tile_pool(name="espool", bufs=6) as espool,
            tc.tile_pool(name="ostp", bufs=3) as ostp,
            tc.tile_pool(name="pp", bufs=2, space="PSUM") as pp,
            tc.tile_pool(name="psS", bufs=2, space="PSUM") as psS,
            tc.tile_pool(name="psy", bufs=2, space="PSUM") as psy,
        ):
            # ---- resident loads: one sync-queue stream in consumption order
            # (per-queue FIFO serializes the transfers, so arrival order ==
            # issue order and nothing competes with the critical early loads)
            fT_sb = res.tile([P, KC * S], BF16, tag="fT", name="fT_sb")
            nc.sync.dma_start(fT_sb[:], fp[:, :])
            wk_t = [res.tile([P, HC], BF16, tag=f"wk{h}", name=f"wk{h}") for h in range(2)]
            nc.sync.dma_start(wk_t[0][:], wkp[:, 0:HC])
            nc.sync.dma_start(wk_t[1][:], wkp[:, HC:2 * HC])
            xs = [res.tile([P, KC * TCH], BF16, tag=f"xs{ch}", name=f"xs{ch}")
                  for ch in range(2)]
            nc.sync.dma_start(xs[0][:], xp0[:, :])
            wq_t = [res.tile([P, HC], BF16, tag=f"wq{h}", name=f"wq{h}") for h in range(2)]
            nc.sync.dma_start(wq_t[0][:], wqp[:, 0:HC])
            nc.sync.dma_start(wq_t[1][:], wqp[:, HC:2 * HC])
            m0_sb = res.tile([P, 2, P], BF16, tag="m0", name="m0_sb")
            nc.sync.dma_start(m0_sb[:], mask0[:, :, :])
            m1_sb = res.tile([P, 2, 2 * P], BF16, tag="m1", name="m1_sb")
            nc.sync.dma_start(m1_sb[:], mask1[:, :, :])
            esel_sb = res.tile([H, C], BF16, tag="esel", name="esel_sb")
            nc.sync.dma_start(esel_sb[:], eselb[:, :])
            wv_t = [res.tile([P, HC], BF16, tag=f"wv{h}", name=f"wv{h}") for h in range(2)]
            nc.sync.dma_start(wv_t[0][:], wvp[:, 0:HC])
            nc.sync.dma_start(wv_t[1][:], wvp[:, HC:2 * HC])
            nc.sync.dma_start(xs[1][:], xp1[:, :])
            wp_t = [res.tile([P, HC], BF16, tag=f"wp{h}", name=f"wp{h}") for h in range(2)]
            nc.sync.dma_start(wp_t[0][:], wpp[:, 0:HC])
            nc.sync.dma_start(wp_t[1][:], wpp[:, HC:2 * HC])

            def fT(k):
                return fT_sb[:, k * S:(k + 1) * S]

            def wfk(m, k):
                return wk_t[m // 4][:, (m % 4) * C + k * P:(m % 4) * C + (k + 1) * P]

            def wq(m, k):
                return wq_t[m // 4][:, (m % 4) * C + k * P:(m % 4) * C + (k + 1) * P]

            def wfv(k):
                return wv_t[k // 4][:, (k % 4) * C:(k % 4 + 1) * C]

            def wp(k):
                return wp_t[k // 4][:, (k % 4) * C:(k % 4 + 1) * C]

            def xT(ch, k):
                return xs[ch][:, k * TCH:(k + 1) * TCH]

            # v_aug[s2]: [s(128), pair(8), parity(2), col(128)] bf16
            # even head (parity 0): cols 0:64 = v, col 64 = 1  -> y at psum
            #   rows 0:64, denominator at row 64
            # odd  head (parity 1): col 0 = 1, cols 64:128 = v -> denominator
            #   at psum row 0, y at rows 64:128
            v_aug = [
                res.tile([P, NP, 2, P], BF16, tag=f"vaug{s2}", name=f"vaug{s2}")
                for s2 in range(2)
            ]
            for s2 in range(2):
                nc.vector.memset(v_aug[s2][:], 0.0)
                nc.sync.dma_start(
                    v_aug[s2][:, :, 0:1, 64:65], onesb[:, :, None, None]
                )
                nc.sync.dma_start(
                    v_aug[s2][:, :, 1:2, 0:1], onesb[:, :, None, None]
                )

            # ---- k projection: kT[m] [128, S] ------------------------------
            kTs = [res.tile([P, S], BF16, tag=f"kT{m}", name=f"kT{m}") for m in range(KC)]
            for m in range(KC):
                ps = pp.tile([P, S], F32, tag="pp", name=f"k_ps{m}")
                for k in range(KC):
                    nc.tensor.matmul(
                        ps[:], wfk(m, k), fT(k),
                        start=(k == 0), stop=(k == KC - 1),
                    )
                nc.vector.tensor_copy(kTs[m][:], ps[:])

            qT = [
                [res.tile([P, TCH], BF16, tag=f"qT{ch}_{m}", name=f"qT{ch}_{m}")
                 for m in range(KC)]
                for ch in range(NCH)
            ]
            yT = [
                [res.tile([P, TCH], BF16, tag=f"yT{ch}_{p}", name=f"yT{ch}_{p}")
                 for p in range(NP)]
                for ch in range(NCH)
            ]
            rstage = [
                res.tile([P, NP * TCH], F32, tag=f"rstg{ch}", name=f"rstg{ch}")
                for ch in range(NCH)
            ]
            r_sb = [
                res.tile([H, TCH], F32, tag=f"rsb{ch}", name=f"rsb{ch}")
                for ch in range(NCH)
            ]
            recf = [
                res.tile([H, TCH], F32, tag=f"recf{ch}", name=f"recf{ch}")
                for ch in range(NCH)
            ]
            rec = [
                res.tile([H, TCH], BF16, tag=f"rec{ch}", name=f"rec{ch}")
                for ch in range(NCH)
            ]

            def qproj(ch):
                for m in range(KC):
                    ps = pp.tile([P, TCH], F32, tag="pp", name=f"q_ps{ch}_{m}")
                    for k in range(KC):
                        nc.tensor.matmul(
                            ps[:], wq(m, k), xT(ch, k),
                            start=(k == 0), stop=(k == KC - 1),
                        )
                    nc.vector.tensor_copy(qT[ch][m][:], ps[:])

            def vproj():
                for s2 in range(2):
                    for cc in range(2):
                        ps = pp.tile([P, 4, 2, D], F32, tag="pp", name=f"v_ps{s2}_{cc}")
                        for k in range(KC):
                            nc.tensor.matmul(
                                ps[:], fT(k)[:, s2 * P:(s2 + 1) * P],
                                wfv(k)[:, c512(cc)],
                                start=(k == 0), stop=(k == KC - 1),
                            )
                        nc.vector.tensor_copy(
                            v_aug[s2][:, cc * 4:(cc + 1) * 4, 0:1, 0:D],
                            ps[:, :, 0:1, :],
                        )
                        nc.vector.tensor_copy(
                            v_aug[s2][:, cc * 4:(cc + 1) * 4, 1:2, D:2 * D],
                            ps[:, :, 1:2, :],
                        )

            def attention(ch):
                for p in range(NP):
                    es = [
                        espool.tile([P, 2, TCH], BF16, tag="es", name=f"es{ch}_{p}_{s2}")
                        for s2 in range(2)
                    ]
                    for s2 in range(2):
                        ps = psS.tile(
                            [P, 2 * TCH], F32, tag="psS", name=f"s_ps{ch}_{p}_{s2}"
                        )
                        nc.tensor.matmul(
                            ps[:, 0:TCH],
                            kTs[p][0:D, s2 * P:(s2 + 1) * P],
                            qT[ch][p][0:D, :],
                            start=True, stop=True,
                        )
                        nc.tensor.matmul(
                            ps[:, TCH:2 * TCH],
                            kTs[p][D:P, s2 * P:(s2 + 1) * P],
                            qT[ch][p][D:P, :],
                            start=True, stop=True,
                        )
                        nc.scalar.activation(
                            es[s2][:], ps[:],
                            mybir.ActivationFunctionType.Exp, scale=0.125,
                        )
                    if ch == 0:
                        nc.vector.tensor_mul(
                            es[0][:, :, 0:P], es[0][:, :, 0:P], m0_sb[:]
                        )
                        nc.vector.tensor_mul(
                            es[1][:, :, 0:2 * P], es[1][:, :, 0:2 * P], m1_sb[:]
                        )
                    psa = psy.tile([P, TCH], F32, tag="psy", name=f"ya{ch}_{p}")
                    psb = psy.tile([P, TCH], F32, tag="psy", name=f"yb{ch}_{p}")
                    for s2 in range(2):
                        nc.tensor.matmul(
                            psa[:], v_aug[s2][:, p:p + 1, 0:1, :],
                            es[s2][:, 0:1, :],
                            start=(s2 == 0), stop=(s2 == 1),
                        )
                    for s2 in range(2):
                        nc.tensor.matmul(
                            psb[:], v_aug[s2][:, p:p + 1, 1:2, :],
                            es[s2][:, 1:2, :],
                            start=(s2 == 0), stop=(s2 == 1),
                        )
                    pcols = slice(p * TCH, (p + 1) * TCH)
                    nc.vector.tensor_copy(yT[ch][p][0:D, :], psa[0:D, :])
                    nc.scalar.copy(rstage[ch][64:65, pcols], psa[64:65, :])
                    nc.vector.tensor_copy(yT[ch][p][D:P, :], psb[D:P, :])
                    nc.vector.tensor_copy(rstage[ch][0:1, pcols], psb[0:1, :])
                    # gather denom rows: r_sb rows 0:8 = odd heads, 8:16 = even
                    nc.sync.dma_start(
                        r_sb[ch][8 + p:9 + p, :], rstage[ch][64:65, pcols]
                    )
                    nc.sync.dma_start(
                        r_sb[ch][p:p + 1, :], rstage[ch][0:1, pcols]
                    )

            def norm(ch):
                nc.vector.reciprocal_approx_fast(recf[ch][:], r_sb[ch][:])
                nc.vector.tensor_copy(rec[ch][:], recf[ch][:])
                for p in range(NP):
                    rb = pp.tile([P, TCH], F32, tag="pp", name=f"rb{ch}_{p}")
                    nc.tensor.matmul(
                        rb[:], esel_sb[:, p * P:(p + 1) * P], rec[ch][:],
                        start=True, stop=True,
                    )
                    nc.vector.tensor_mul(yT[ch][p][:], yT[ch][p][:], rb[:])

            def outproj(ch):
                for mm in range(4):
                    m = ch * 4 + mm
                    for cc in range(2):
                        ps = pp.tile([P, 512], F32, tag="pp", name=f"o_ps{m}_{cc}")
                        for k in range(KC):
                            nc.tensor.matmul(
                                ps[:], yT[ch][k][:, mm * P:(mm + 1) * P],
                                wp(k)[:, c512(cc)],
                                start=(k == 0), stop=(k == KC - 1),
                            )
                        ost = ostp.tile([P, 512], F32, tag="ost", name=f"ost{m}_{cc}")
                        if cc == 0:
                            nc.scalar.copy(ost[:], ps[:])
                        else:
                            nc.vector.tensor_copy(ost[:], ps[:])
                        nc.sync.dma_start(out[m * P:(m + 1) * P, c512(cc)], ost[:])

            kproj_done = None  # emission order below drives scheduler priority
            qproj(0)
            vproj()
            attention(0)
            qproj(1)
            norm(0)
            attention(1)
            outproj(0)
            norm(1)
            outproj(1)

    nc.finalize()
    return nc


def _host_prep_fast(x, feature, Wq, Wf, Wp):
    import ml_dtypes

    bf = ml_dtypes.bfloat16
    f32 = np.float32

    def pack_colblocks(W):
        # pack[p, m*C + k*128 + c] = W.T[k*128+p, m*128+c]
        WT = np.ascontiguousarray(np.asarray(W, f32).T)
        return np.ascontiguousarray(
            WT.reshape(KC, P, KC, P).transpose(1, 2, 0, 3).reshape(P, KC * C).astype(bf)
        )

    def pack_rows(M2d, X):
        # pack[p, k*X + t] = M2d[k*128+p, t]
        return np.ascontiguousarray(
            np.asarray(M2d, f32).reshape(KC, P, X).transpose(1, 0, 2)
            .reshape(P, KC * X).astype(bf)
        )

    i = np.arange(P)[:, None]
    j = np.arange(P)[None, :]
    m0 = (j >= i).astype(f32)
    j2 = np.arange(2 * P)[None, :]
    m1 = (j2 >= (P + i)).astype(f32)

    hsel = np.empty((H,), np.int64)
    hsel[:NP] = 2 * np.arange(NP) + 1
    hsel[NP:] = 2 * np.arange(NP)
    col = np.arange(C)[None, :]
    esel = (hsel[:, None] == col // D).astype(f32)

    shared = {
        "wqp": pack_colblocks(Wq),
        "wkp": pack_colblocks(Wf[:C]),
        "wvp": pack_rows(np.asarray(Wf[C:], f32).T, C),
        "wpp": pack_rows(np.asarray(Wp, f32).T, C),
        "mask0": np.ascontiguousarray(
            np.broadcast_to(m0[:, None, :], (P, 2, P)).astype(bf)
        ),
        "mask1": np.ascontiguousarray(
            np.broadcast_to(m1[:, None, :], (P, 2, 2 * P)).astype(bf)
        ),
        "onesb": np.ones((P, NP), bf),
        "eselb": np.ascontiguousarray(esel.astype(bf)),
    }
    in_maps = []
    for b in range(B):
        m = dict(shared)
        xT_b = np.asarray(x[b], f32).T
        m["xp0"] = pack_rows(xT_b[:, 0:TCH], TCH)
        m["xp1"] = pack_rows(xT_b[:, TCH:T], TCH)
        m["fp"] = pack_rows(np.asarray(feature[b], f32).T, S)
        in_maps.append(m)
    return in_maps


# --------------------------------------------------------------------------
# slow path (original f32r kernel; used only when LoRA B / bias are nonzero)
# --------------------------------------------------------------------------

def _build_slow(flags):
    has_lq, has_lf, has_lp, has_bq, has_bfk, has_bfv, has_bp = flags
    nc = bacc.Bacc("TRN2", target_bir_lowering=False, debug=False)

    xT = nc.declare_dram_parameter("xT", [C, T], F32R, isOutput=False)
    fT = nc.declare_dram_parameter("fT", [C, S], F32R, isOutput=False)
    WqT = nc.declare_dram_parameter("WqT", [C, C], F32R, isOutput=False)
    WfkT = nc.declare_dram_parameter("WfkT", [C, C], F32R, isOutput=False)
    WfvT = nc.declare_dram_parameter("WfvT", [C, C], F32R, isOutput=False)
    WpT = nc.declare_dram_parameter("WpT", [C, C], F32R, isOutput=False)
    mask = nc.declare_dram_parameter("mask", [P, 384], F32R, isOutput=False)
    vones = nc.declare_dram_parameter("vones", [P, H], F32R, isOutput=False)
    Esel = nc.declare_dram_parameter("Esel", [H, C], F32R, isOutput=False)
    if has_lq:
        AqT = nc.declare_dram_parameter("AqT", [C, R], F32R, isOutput=False)
        BqTs = nc.declare_dram_parameter("BqTs", [R, C], F32R, isOutput=False)
    if has_lf:
        AfT = nc.declare_dram_parameter("AfT", [C, R], F32R, isOutput=False)
        BfkTs = nc.declare_dram_parameter("BfkTs", [R, C], F32R, isOutput=False)
        BfvTs = nc.declare_dram_parameter("BfvTs", [R, C], F32R, isOutput=False)
    if has_lp:
        ApT = nc.declare_dram_parameter("ApT", [C, R], F32R, isOutput=False)
        BpTs = nc.declare_dram_parameter("BpTs", [R, C], F32R, isOutput=False)
    if has_bq:
        bq_pp = nc.declare_dram_parameter("bq_pp", [P, KC], F32, isOutput=False)
    if has_bfk:
        bfk_pp = nc.declare_dram_parameter("bfk_pp", [P, KC], F32, isOutput=False)
    if has_bfv:
        bfv_row = nc.declare_dram_parameter("bfv_row", [1, C], F32R, isOutput=False)
    if has_bp:
        bp_row = nc.declare_dram_parameter("bp_row", [1, C], F32R, isOutput=False)
    out = nc.declare_dram_parameter("out", [T, C], F32, isOutput=True)

    xT3 = xT.rearrange("(ko p) t -> ko p t", p=P)
    fT3 = fT.rearrange("(ko p) s -> ko p s", p=P)
    WqT3 = WqT.rearrange("(ko p) c -> p ko c", p=P)
    WfkT3 = WfkT.rearrange("(ko p) c -> p ko c", p=P)
    WfvT3 = WfvT.rearrange("(ko p) c -> ko p c", p=P)
    WpT3 = WpT.rearrange("(ko p) c -> ko p c", p=P)

    def c512(i):
        return slice(i * 512, (i + 1) * 512)

    with tile.TileContext(nc) as tc:
        with (
            tc.tile_pool(name="big", bufs=8) as big,
            tc.tile_pool(name="qpool", bufs=8) as qpool,
            tc.tile_pool(name="small", bufs=1) as small,
            tc.tile_pool(name="wcol", bufs=3) as wcol,
            tc.tile_pool(name="wrow", bufs=8) as wrow,
            tc.tile_pool(name="expp", bufs=6) as expp,
            tc.tile_pool(name="stg", bufs=3) as stg,
            tc.tile_pool(name="psA", bufs=4, space="PSUM") as psA,
            tc.tile_pool(name="psB", bufs=2, space="PSUM") as psB,
        ):
            xTs = [big.tile([P, T], F32R, tag="big", name=f"xT{k}") for k in range(KC)]
            for k in range(KC):
                nc.sync.dma_start(xTs[k][:], xT3[k])
            fTs = [small.tile([P, S], F32R, tag=f"fT{k}", name=f"fT{k}") for k in range(KC)]
            for k in range(KC):
                nc.sync.dma_start(fTs[k][:], fT3[k])
            mask_sb = small.tile([P, 384], F32R, tag="mask", name="mask_sb")
            nc.sync.dma_start(mask_sb[:], mask[:, :])
            esel_sb = small.tile([H, C], F32R, tag="esel", name="esel_sb")
            nc.sync.dma_start(esel_sb[:], Esel[:, :])
            if has_lq:
                aq_sb = small.tile([P, KC, R], F32R, tag="aq", name="aq_sb")
                nc.sync.dma_start(aq_sb[:], AqT.rearrange("(ko p) r -> p ko r", p=P))
                bqs_sb = small.tile([R, C], F32R, tag="bqs", name="bqs_sb")
                nc.sync.dma_start(bqs_sb[:], BqTs[:, :])
            if has_lf:
                af_sb = small.tile([P, KC, R], F32R, tag="af", name="af_sb")
                nc.sync.dma_start(af_sb[:], AfT.rearrange("(ko p) r -> p ko r", p=P))
                bfks_sb = small.tile([R, C], F32R, tag="bfks", name="bfks_sb")
                nc.sync.dma_start(bfks_sb[:], BfkTs[:, :])
                bfvs_sb = small.tile([R, C], F32R, tag="bfvs", name="bfvs_sb")
                nc.sync.dma_start(bfvs_sb[:], BfvTs[:, :])
            if has_lp:
                ap_sb = small.tile([P, KC, R], F32R, tag="ap", name="ap_sb")
                nc.sync.dma_start(ap_sb[:], ApT.rearrange("(ko p) r -> p ko r", p=P))
                bps_sb = small.tile([R, C], F32R, tag="bps", name="bps_sb")
                nc.sync.dma_start(bps_sb[:], BpTs[:, :])
            if has_bq:
                bq_sb = small.tile([P, KC], F32, tag="bq", name="bq_sb")
                nc.sync.dma_start(bq_sb[:], bq_pp[:, :])
            if has_bfk:
                bfk_sb = small.tile([P, KC], F32, tag="bfk", name="bfk_sb")
                nc.sync.dma_start(bfk_sb[:], bfk_pp[:, :])
            if has_bfv or has_bp:
                ones1 = small.tile([1, P], F32R, tag="ones1", name="ones1")
                nc.sync.dma_start(ones1[:], vones.rearrange("p h -> (p h)")[None, 0:P])
            if has_bfv:
                bfv_sb = small.tile([1, C], F32R, tag="bfv", name="bfv_sb")
                nc.sync.dma_start(bfv_sb[:], bfv_row[:, :])
            if has_bp:
                bp_sb = small.tile([1, C], F32R, tag="bp", name="bp_sb")
                nc.sync.dma_start(bp_sb[:], bp_row[:, :])

            if has_lq:
                ups = psB.tile([P, T], F32, tag="y", name="uq_ps")
                for ch in range(NCH):
                    for k in range(KC):
                        nc.tensor.matmul(
                            ups[:R, c512(ch)], aq_sb[:, k, :], xTs[k][:, c512(ch)],
                            start=(k == 0), stop=(k == KC - 1),
                        )
                uq_sb = small.tile([R, T], F32R, tag="uq", name="uq_sb")
                nc.scalar.copy(uq_sb[:], ups[:R, :])
            if has_lf:
                ufs = psB.tile([P, T], F32, tag="y", name="uf_ps")
                for k in range(KC):
                    nc.tensor.matmul(
                        ufs[:R, :S], af_sb[:, k, :], fTs[k][:],
                        start=(k == 0), stop=(k == KC - 1),
                    )
                uf_sb = small.tile([R, S], F32R, tag="uf", name="uf_sb")
                nc.scalar.copy(uf_sb[:], ufs[:R, :S])

            kTs = [small.tile([P, S], F32R, tag=f"kT{m}", name=f"kT{m}") for m in range(KC)]
            for m in range(KC):
                wk_m = wcol.tile([P, KC, P], F32R, tag="wcol", name=f"wk{m}")
                nc.sync.dma_start(wk_m[:], WfkT3[:, :, m * P:(m + 1) * P])
                ps = psA.tile([P, S], F32, tag="mm", name=f"k_ps{m}")
                for k in range(KC):
                    nc.tensor.matmul(
                        ps[:], wk_m[:, k, :], fTs[k][:],
                        start=(k == 0), stop=(k == KC - 1 and not has_lf),
                    )
                if has_lf:
                    nc.tensor.matmul(
                        ps[:], bfks_sb[:, m * P:(m + 1) * P], uf_sb[:],
                        start=False, stop=True,
                    )
                if has_bfk:
                    nc.scalar.activation(
                        kTs[m][:], ps[:], mybir.ActivationFunctionType.Identity,
                        bias=bfk_sb[:, m:m + 1], scale=1.0,
                    )
                else:
                    nc.vector.tensor_copy(kTs[m][:], ps[:])

            wfv = [wrow.tile([P, C], F32R, tag="wrow", name=f"wfv{k}") for k in range(KC)]
            for k in range(KC):
                nc.sync.dma_start(wfv[k][:], WfvT3[k])
            v_aug = [
                small.tile([P, H, D + 1], F32R, tag=f"vaug{s2}", name=f"vaug{s2}")
                for s2 in range(2)
            ]
            for s2 in range(2):
                nc.sync.dma_start(v_aug[s2][:, :, D], vones[:, :])
                for ch in range(NCH):
                    ps = psA.tile([P, 512], F32, tag="mm", name=f"v_ps{s2}_{ch}")
                    nmm = KC + (1 if has_lf else 0) + (1 if has_bfv else 0)
                    i = 0
                    for k in range(KC):
                        i += 1
                        nc.tensor.matmul(
                            ps[:], fTs[k][:, s2 * P:(s2 + 1) * P],
                            wfv[k][:, c512(ch)],
                            start=(i == 1), stop=(i == nmm),
                        )
                    if has_lf:
                        i += 1
                        nc.tensor.matmul(
                            ps[:], uf_sb[:, s2 * P:(s2 + 1) * P],
                            bfvs_sb[:, c512(ch)], start=False, stop=(i == nmm),
                        )
                    if has_bfv:
                        i += 1
                        nc.tensor.matmul(
                            ps[:], ones1[:], bfv_sb[:, c512(ch)],
                            start=False, stop=(i == nmm),
                        )
                    for hh in range(8):
                        h = ch * 8 + hh
                        nc.vector.tensor_copy(
                            v_aug[s2][:, h, 0:D], ps[:, hh * D:(hh + 1) * D]
                        )

            qTs = [qpool.tile([P, T], F32R, tag="qT", name=f"qT{m}") for m in range(MT)]
            for m in range(KC):
                wq_m = wcol.tile([P, KC, P], F32R, tag="wcol", name=f"wq{m}")
                nc.sync.dma_start(wq_m[:], WqT3[:, :, m * P:(m + 1) * P])
                for ch in range(NCH):
                    ps = psA.tile([P, 512], F32, tag="mm", name=f"q_ps{m}_{ch}")
                    for k in range(KC):
                        nc.tensor.matmul(
                            ps[:], wq_m[:, k, :], xTs[k][:, c512(ch)],
                            start=(k == 0), stop=(k == KC - 1 and not has_lq),
                        )
                    if has_lq:
                        nc.tensor.matmul(
                            ps[:], bqs_sb[:, m * P:(m + 1) * P], uq_sb[:, c512(ch)],
                            start=False, stop=True,
                        )
                    if has_bq:
                        nc.scalar.activation(
                            qTs[m][:, c512(ch)], ps[:],
                            mybir.ActivationFunctionType.Identity,
                            bias=bq_sb[:, m:m + 1], scale=1.0,
                        )
                    else:
                        nc.vector.tensor_copy(qTs[m][:, c512(ch)], ps[:])

            yTr = [big.tile([P, T], F32R, tag="big", name=f"yTr{p}") for p in range(KC)]
            r_sb = small.tile([H, T], F32R, tag="rsum", name="r_sb")
            for h in range(H):
                m, off = h // 2, (h % 2) * D
                kt_h = kTs[m][off:off + D, :]
                qt_h = qTs[m][off:off + D, :]
                es = [expp.tile([P, T], F32R, tag="exp", name=f"e{h}_{s2}") for s2 in range(2)]
                for s2 in range(2):
                    for ch in range(NCH):
                        ps = psA.tile([P, 512], F32, tag="mm", name=f"s_ps{h}_{s2}_{ch}")
                        nc.tensor.matmul(
                            ps[:], kt_h[:, s2 * P:(s2 + 1) * P], qt_h[:, c512(ch)],
                            start=True, stop=True,
                        )
                        nc.scalar.activation(
                            es[s2][:, c512(ch)], ps[:],
                            mybir.ActivationFunctionType.Exp, scale=0.125,
                        )
                nc.vector.tensor_mul(es[0][:, 0:P], es[0][:, 0:P], mask_sb[:, 0:P])
                nc.vector.tensor_mul(es[1][:, 0:S], es[1][:, 0:S], mask_sb[:, P:384])
                psy = psB.tile([P, T], F32, tag="y", name=f"y_ps{h}")
                for ch in range(NCH):
                    for s2 in range(2):
                        nc.tensor.matmul(
                            psy[:D + 1, c512(ch)], v_aug[s2][:, h, :],
                            es[s2][:, c512(ch)], start=(s2 == 0), stop=(s2 == 1),
                        )
                st = stg.tile([P, T], F32R, tag="hstage", name=f"st{h}")
                if off == 0:
                    nc.vector.tensor_copy(yTr[m][0:D, :], psy[0:D, :])
                    nc.vector.tensor_copy(st[D:D + 1, :], psy[D:D + 1, :])
                else:
                    nc.vector.tensor_copy(st[0:D + 1, :], psy[0:D + 1, :])
                    nc.sync.dma_start(yTr[m][off:off + D, :], st[0:D, :])
                nc.sync.dma_start(r_sb[h:h + 1, :], st[D:D + 1, :])

            recf = small.tile([H, T], F32, tag="recf", name="recf")
            nc.vector.reciprocal(recf[:], r_sb[:])
            rec = small.tile([H, T], F32R, tag="rec", name="rec")
            nc.vector.tensor_copy(rec[:], recf[:])
            for p in range(KC):
                rb = psB.tile([P, T], F32, tag="y", name=f"rb{p}")
                for ch in range(NCH):
                    nc.tensor.matmul(
                        rb[:, c512(ch)], esel_sb[:, p * P:(p + 1) * P],
                        rec[:, c512(ch)], start=True, stop=True,
                    )
                nc.vector.tensor_mul(yTr[p][:], yTr[p][:], rb[:])

            if has_lp:
                upsd = psB.tile([P, T], F32, tag="y", name="up_ps")
                for ch in range(NCH):
                    for k in range(KC):
                        nc.tensor.matmul(
                            upsd[:R, c512(ch)], ap_sb[:, k, :], yTr[k][:, c512(ch)],
                            start=(k == 0), stop=(k == KC - 1),
                        )
                up_sb = small.tile([R, T], F32R, tag="up", name="up_sb")
                nc.scalar.copy(up_sb[:], upsd[:R, :])
            wp = [wrow.tile([P, C], F32R, tag="wrow", name=f"wp{k}") for k in range(KC)]
            for k in range(KC):
                nc.sync.dma_start(wp[k][:], WpT3[k])
            for m in range(MT):
                for ch in range(NCH):
                    ps = psA.tile([P, 512], F32, tag="mm", name=f"o_ps{m}_{ch}")
                    nmm = KC + (1 if has_lp else 0) + (1 if has_bp else 0)
                    i = 0
                    for k in range(KC):
                        i += 1
                        nc.tensor.matmul(
                            ps[:], yTr[k][:, m * P:(m + 1) * P], wp[k][:, c512(ch)],
                            start=(i == 1), stop=(i == nmm),
                        )
                    if has_lp:
                        i += 1
                        nc.tensor.matmul(
                            ps[:], up_sb[:, m * P:(m + 1) * P], bps_sb[:, c512(ch)],
                            start=False, stop=(i == nmm),
                        )
                    if has_bp:
                        i += 1
                        nc.tensor.matmul(
                            ps[:], ones1[:], bp_sb[:, c512(ch)],
                            start=False, stop=(i == nmm),
                        )
                    ost = wcol.tile([P, 512], F32, tag="ostage", name=f"ost{m}_{ch}")
                    nc.scalar.copy(ost[:], ps[:])
                    nc.sync.dma_start(out[m * P:(m + 1) * P, c512(ch)], ost[:])

    nc.finalize()
    return nc


def _host_prep_slow(x, feature, Wq, bq, Aq, Bq, Wf, bf, Af, Bf, Wp, bp, Ap, Bp, flags):
    f32 = np.float32
    shared = {
        "WqT": np.ascontiguousarray(np.asarray(Wq, f32).T),
        "WfkT": np.ascontiguousarray(np.asarray(Wf[:C], f32).T),
        "WfvT": np.ascontiguousarray(np.asarray(Wf[C:], f32).T),
        "WpT": np.ascontiguousarray(np.asarray(Wp, f32).T),
    }
    i = np.arange(P)[:, None]
    j = np.arange(384)[None, :]
    m0 = (j[:, :P] >= i).astype(f32)
    m1 = ((j[:, P:384] - P) >= (P + i)).astype(f32)
    shared["mask"] = np.ascontiguousarray(np.concatenate([m0, m1], axis=1))
    shared["vones"] = np.ones((P, H), f32)
    hsel = np.arange(H)[:, None]
    col = np.arange(C)[None, :]
    shared["Esel"] = np.ascontiguousarray((hsel == col // D).astype(f32))
    has_lq, has_lf, has_lp, has_bq, has_bfk, has_bfv, has_bp = flags
    if has_lq:
        shared["AqT"] = np.ascontiguousarray(np.asarray(Aq, f32).T)
        shared["BqTs"] = np.ascontiguousarray(np.asarray(Bq, f32).T * SCALING)
    if has_lf:
        shared["AfT"] = np.ascontiguousarray(np.asarray(Af, f32).T)
        shared["BfkTs"] = np.ascontiguousarray(np.asarray(Bf[:C], f32).T * SCALING)
        shared["BfvTs"] = np.ascontiguousarray(np.asarray(Bf[C:], f32).T * SCALING)
    if has_lp:
        shared["ApT"] = np.ascontiguousarray(np.asarray(Ap, f32).T)
        shared["BpTs"] = np.ascontiguousarray(np.asarray(Bp, f32).T * SCALING)
    if has_bq:
        shared["bq_pp"] = np.ascontiguousarray(np.asarray(bq, f32).reshape(KC, P).T)
    if has_bfk:
        shared["bfk_pp"] = np.ascontiguousarray(np.asarray(bf[:C], f32).reshape(KC, P).T)
    if has_bfv:
        shared["bfv_row"] = np.ascontiguousarray(np.asarray(bf[C:], f32).reshape(1, C))
    if has_bp:
        shared["bp_row"] = np.ascontiguousarray(np.asarray(bp, f32).reshape(1, C))

    in_maps = []
    for b in range(B):
        m = dict(shared)
        m["xT"] = np.ascontiguousarray(np.asarray(x[b], f32).T)
        m["fT"] = np.ascontiguousarray(np.asarray(feature[b], f32).T)
        in_maps.append(m)
    return in_maps


# --------------------------------------------------------------------------
# dispatch
# --------------------------------------------------------------------------

def _run(inputs, trace=False, **spmd_kwargs):
    x, feature = inputs["x"], inputs["feature"]
    Wq, bq, Aq, Bq = inputs["Wq"], inputs["bq"], inputs["Aq"], inputs["Bq"]
    Wf, bf_, Af, Bf = inputs["Wf"], inputs["bf"], inputs["Af"], inputs["Bf"]
    Wp, bp, Ap, Bp = inputs["Wp"], inputs["bp"], inputs["Ap"], inputs["Bp"]
    flags = (
        bool(np.any(Bq)), bool(np.any(Bf)), bool(np.any(Bp)),
        bool(np.any(bq)), bool(np.any(bf_[:C])), bool(np.any(bf_[C:])),
        bool(np.any(bp)),
    )
    if any(flags):
        key = ("slow", flags)
        nc = _nc_cache.get(key)
        if nc is None:
            nc = _build_slow(flags)
            _nc_cache[key] = nc
        in_maps = _host_prep_slow(
            x, feature, Wq, bq, Aq, Bq, Wf, bf_, Af, Bf, Wp, bp, Ap, Bp, flags
        )
    else:
        key = "fast"
        nc = _nc_cache.get(key)
        if nc is None:
            nc = _build_fast()
            _nc_cache[key] = nc
        in_maps = _host_prep_fast(x, feature, Wq, Wf, Wp)
    res = run_bass_kernel_spmd(
        nc, in_maps, core_ids=list(range(B)), trace=trace, **spmd_kwargs
    )
    out = np.stack([res.results[b]["out"] for b in range(B)], axis=0)
    return out, res


def kernel(**inputs):
    out, _ = _run(inputs, trace=False)
    return out


# revision 14
# speedup vs baseline: 1.2001x; 1.1312x over previous
"""Trainium2 Bass kernel for CrossAttention with LoRA.

Data-parallel over batch (B=8 -> 8 NeuronCores, one batch element per core).
No collectives.

Fast path (the actual case: loralib-init B matrices and biases are all zero,
so LoRA/bias terms vanish): fully fused bf16 cross-attention with a T-chunked
pipeline -- see _build_fast. Key tricks:
  * all matmul operands bf16 (halves DMA/SBUF; PE cost unchanged; psum f32)
  * score matmuls for an even/odd head pair issued at PE row-tile positions
    (0,0)/(64,0) -- K=64 each, run concurrently in the PE array
  * attn@v uses a zero-padded M=128 stationary per head:
      even head [v(64) | 1 | 0...]: y -> psum rows 0:64, denom -> row 64
      odd  head [0... | 1 | v(64)]: denom -> row 63, y -> rows 64:128
    so every psum drain is partition-aligned (no SBUF->SBUF bounce)
  * attention is chunked over T (2 x 512): chunk 0's softmax-normalize and
    output projection overlap chunk 1's attention; q-projection of chunk 1
    fills PE gaps during chunk 0's attention (Tile's list scheduler pops
    ready work in emission order)
  * weights DMA'd in consumption order, packed contiguously per-tile on host

Slow path (any nonzero LoRA B / bias): the original f32r kernel, kept
verbatim for correctness insurance.
"""

import numpy as np

import concourse.bass as bass  # noqa: F401  (bass types via bacc)
import concourse.mybir as mybir
import concourse.tile as tile
from concourse import bacc
from concourse.bass_utils import run_bass_kernel_spmd

B, T, S, C, H, D, R = 8, 1024, 256, 1024, 16, 64, 16
SCALING = 1.0 / 16.0
P = 128
KC = C // P  # 8 k-tiles over the embedding dim
MT = T // P  # 8 tiles over T
NP = H // 2  # 8 head pairs
NCH = 2      # t-chunks (fast path) / psum chunks (slow path)
TCH = T // NCH
F32 = mybir.dt.float32
F32R = mybir.dt.float32r
BF16 = mybir.dt.bfloat16

_nc_cache: dict = {}


# --------------------------------------------------------------------------
# fast path
# --------------------------------------------------------------------------

def _build_fast():
    nc = bacc.Bacc("TRN2", target_bir_lowering=False, debug=False)

    # host-packed contiguous per-partition layouts:
    #   xp{ch}[p, k*TCH + t] = x.T[k*128+p, ch*TCH+t]
    #   fp[p, k*S + s]       = f.T[k*128+p, s]
    #   wqp/wkp[p, m*C + k*128 + c] = W.T[k*128+p, m*128+c]   (m-major)
    #   wvp/wpp[p, k*C + c]  = W.T[k*128+p, c]                (k-major)
    xp = nc.declare_dram_parameter("xp", [P, KC * T], BF16, isOutput=False)
    fp = nc.declare_dram_parameter("fp", [P, KC * S], BF16, isOutput=False)
    wqp = nc.declare_dram_parameter("wqp", [P, KC * C], BF16, isOutput=False)
    wkp = nc.declare_dram_parameter("wkp", [P, KC * C], BF16, isOutput=False)
    wvp = nc.declare_dram_parameter("wvp", [P, KC * C], BF16, isOutput=False)
    wpp = nc.declare_dram_parameter("wpp", [P, KC * C], BF16, isOutput=False)
    mask0 = nc.declare_dram_parameter("mask0", [P, 2, P], BF16, isOutput=False)
    mask1 = nc.declare_dram_parameter("mask1", [P, 2, 2 * P], BF16, isOutput=False)
    onesb = nc.declare_dram_parameter("onesb", [P, NP], BF16, isOutput=False)
    eselb = nc.declare_dram_parameter("eselb", [H, C], BF16, isOutput=False)
    out = nc.declare_dram_parameter("out", [T, C], F32, isOutput=True)

    HC = 4 * C

    def c512(i):
        return slice(i * 512, (i + 1) * 512)

    with tile.TileContext(nc) as tc:
        with (
            tc.tile_pool(name="res", bufs=1) as res,
            tc.tile_pool(name="espool", bufs=6) as espool,
            tc.tile_pool(name="ostp", bufs=3) as ostp,
            tc.tile_pool(name="pp", bufs=2, space="PSUM") as pp,
            tc.tile_pool(name="psS", bufs=2, space="PSUM") as psS,
            tc.tile_pool(name="psy", bufs=2, space="PSUM") as psy,
        ):
            # ---- resident loads: one sync-queue stream in consumption order
            # (per-queue FIFO serializes the transfers, so arrival order ==
            # issue order and nothing competes with the critical early loads)
            # per-piece 2KB/partition tiles; one serialized sync-queue
            # stream in consumption order (per-queue FIFO => arrival order)
            fTs = [res.tile([P, S], BF16, tag=f"fT{k}", name=f"fT{k}") for k in range(KC)]
            wks = [res.tile([P, C], BF16, tag=f"wk{m}", name=f"wk{m}") for m in range(KC)]
            wqs = [res.tile([P, C], BF16, tag=f"wq{m}", name=f"wq{m}") for m in range(KC)]
            xts = [res.tile([P, T], BF16, tag=f"xt{k}", name=f"xt{k}") for k in range(KC)]
            wvs = [res.tile([P, C], BF16, tag=f"wv{k}", name=f"wv{k}") for k in range(KC)]
            wps = [res.tile([P, C], BF16, tag=f"wp{k}", name=f"wp{k}") for k in range(KC)]
            for k in range(KC):
                nc.sync.dma_start(fTs[k][:], fp[:, k * S:(k + 1) * S])
            for m in range(KC):
                nc.sync.dma_start(wks[m][:], wkp[:, m * C:(m + 1) * C])
            for m in range(KC):
                nc.sync.dma_start(wqs[m][:], wqp[:, m * C:(m + 1) * C])
                nc.sync.dma_start(xts[m][:], xp[:, m * T:(m + 1) * T])
            m0_sb = res.tile([P, 2, P], BF16, tag="m0", name="m0_sb")
            nc.sync.dma_start(m0_sb[:], mask0[:, :, :])
            m1_sb = res.tile([P, 2, 2 * P], BF16, tag="m1", name="m1_sb")
            nc.sync.dma_start(m1_sb[:], mask1[:, :, :])
            esel_sb = res.tile([H, C], BF16, tag="esel", name="esel_sb")
            nc.sync.dma_start(esel_sb[:], eselb[:, :])
            for k in range(KC):
                nc.sync.dma_start(wvs[k][:], wvp[:, k * C:(k + 1) * C])
            for k in range(KC):
                nc.sync.dma_start(wps[k][:], wpp[:, k * C:(k + 1) * C])

            def fT(k):
                return fTs[k][:]

            def wfk(m, k):
                return wks[m][:, k * P:(k + 1) * P]

            def wq(m, k):
                return wqs[m][:, k * P:(k + 1) * P]

            def wfv(k):
                return wvs[k][:]

            def wp(k):
                return wps[k][:]

            def xT(ch, k):
                return xts[k][:, ch * TCH:(ch + 1) * TCH]

            # v_aug[s2]: [s(128), pair(8), parity(2), col(128)] bf16
            # even head (parity 0): cols 0:64 = v, col 64 = 1  -> y at psum
            #   rows 0:64, denominator at row 64
            # odd  head (parity 1): col 0 = 1, cols 64:128 = v -> denominator
            #   at psum row 0, y at rows 64:128
            v_aug = [
                res.tile([P, NP, 2, P], BF16, tag=f"vaug{s2}", name=f"vaug{s2}")
                for s2 in range(2)
            ]
            for s2 in range(2):
                nc.vector.memset(v_aug[s2][:], 0.0)
                nc.sync.dma_start(
                    v_aug[s2][:, :, 0:1, 64:65], onesb[:, :, None, None]
                )
                nc.sync.dma_start(
                    v_aug[s2][:, :, 1:2, 0:1], onesb[:, :, None, None]
                )

            # ---- k projection: kT[m] [128, S] ------------------------------
            kTs = [res.tile([P, S], BF16, tag=f"kT{m}", name=f"kT{m}") for m in range(KC)]
            for m in range(KC):
                ps = pp.tile([P, S], F32, tag="pp", name=f"k_ps{m}")
                for k in range(KC):
                    nc.tensor.matmul(
                        ps[:], wfk(m, k), fT(k),
                        start=(k == 0), stop=(k == KC - 1),
                    )
                nc.vector.tensor_copy(kTs[m][:], ps[:])

            qT = [
                [res.tile([P, TCH], BF16, tag=f"qT{ch}_{m}", name=f"qT{ch}_{m}")
                 for m in range(KC)]
                for ch in range(NCH)
            ]
            yT = [
                [res.tile([P, TCH], BF16, tag=f"yT{ch}_{p}", name=f"yT{ch}_{p}")
                 for p in range(NP)]
                for ch in range(NCH)
            ]
            rstage = [
                res.tile([P, NP * TCH], F32, tag=f"rstg{ch}", name=f"rstg{ch}")
                for ch in range(NCH)
            ]
            r_sb = [
                res.tile([H, TCH], F32, tag=f"rsb{ch}", name=f"rsb{ch}")
                for ch in range(NCH)
            ]
            recf = [
                res.tile([H, TCH], F32, tag=f"recf{ch}", name=f"recf{ch}")
                for ch in range(NCH)
            ]
            rec = [
                res.tile([H, TCH], BF16, tag=f"rec{ch}", name=f"rec{ch}")
                for ch in range(NCH)
            ]

            def qproj(ch):
                for m in range(KC):
                    ps = pp.tile([P, TCH], F32, tag="pp", name=f"q_ps{ch}_{m}")
                    for k in range(KC):
                        nc.tensor.matmul(
                            ps[:], wq(m, k), xT(ch, k),
                            start=(k == 0), stop=(k == KC - 1),
                        )
                    nc.vector.tensor_copy(qT[ch][m][:], ps[:])

            def vproj():
                for s2 in range(2):
                    for cc in range(2):
                        ps = pp.tile([P, 4, 2, D], F32, tag="pp", name=f"v_ps{s2}_{cc}")
                        for k in range(KC):
                            nc.tensor.matmul(
                                ps[:], fT(k)[:, s2 * P:(s2 + 1) * P],
                                wfv(k)[:, c512(cc)],
                                start=(k == 0), stop=(k == KC - 1),
                            )
                        nc.vector.tensor_copy(
                            v_aug[s2][:, cc * 4:(cc + 1) * 4, 0:1, 0:D],
                            ps[:, :, 0:1, :],
                        )
                        nc.vector.tensor_copy(
                            v_aug[s2][:, cc * 4:(cc + 1) * 4, 1:2, D:2 * D],
                            ps[:, :, 1:2, :],
                        )

            def attention(ch):
                for p in range(NP):
                    es = [
                        espool.tile([P, 2, TCH], BF16, tag="es", name=f"es{ch}_{p}_{s2}")
                        for s2 in range(2)
                    ]
                    for s2 in range(2):
                        ps = psS.tile(
                            [P, 2 * TCH], F32, tag="psS", name=f"s_ps{ch}_{p}_{s2}"
                        )
                        nc.tensor.matmul(
                            ps[:, 0:TCH],
                            kTs[p][0:D, s2 * P:(s2 + 1) * P],
                            qT[ch][p][0:D, :],
                            start=True, stop=True,
                        )
                        nc.tensor.matmul(
                            ps[:, TCH:2 * TCH],
                            kTs[p][D:P, s2 * P:(s2 + 1) * P],
                            qT[ch][p][D:P, :],
                            start=True, stop=True,
                        )
                        nc.scalar.activation(
                            es[s2][:], ps[:],
                            mybir.ActivationFunctionType.Exp, scale=0.125,
                        )
                    if ch == 0:
                        nc.vector.tensor_mul(
                            es[0][:, :, 0:P], es[0][:, :, 0:P], m0_sb[:]
                        )
                        nc.vector.tensor_mul(
                            es[1][:, :, 0:2 * P], es[1][:, :, 0:2 * P], m1_sb[:]
                        )
                    psa = psy.tile([P, TCH], F32, tag="psy", name=f"ya{ch}_{p}")
                    psb = psy.tile([P, TCH], F32, tag="psy", name=f"yb{ch}_{p}")
                    for s2 in range(2):
                        nc.tensor.matmul(
                            psa[:], v_aug[s2][:, p:p + 1, 0:1, :],
                            es[s2][:, 0:1, :],
                            start=(s2 == 0), stop=(s2 == 1),
                        )
                    for s2 in range(2):
                        nc.tensor.matmul(
                            psb[:], v_aug[s2][:, p:p + 1, 1:2, :],
                            es[s2][:, 1:2, :],
                            start=(s2 == 0), stop=(s2 == 1),
                        )
                    pcols = slice(p * TCH, (p + 1) * TCH)
                    nc.vector.tensor_copy(yT[ch][p][0:D, :], psa[0:D, :])
                    nc.scalar.copy(rstage[ch][64:65, pcols], psa[64:65, :])
                    nc.vector.tensor_copy(yT[ch][p][D:P, :], psb[D:P, :])
                    nc.vector.tensor_copy(rstage[ch][0:1, pcols], psb[0:1, :])
                    # gather denom rows: r_sb rows 0:8 = odd heads, 8:16 = even
                    nc.sync.dma_start(
                        r_sb[ch][8 + p:9 + p, :], rstage[ch][64:65, pcols]
                    )
                    nc.sync.dma_start(
                        r_sb[ch][p:p + 1, :], rstage[ch][0:1, pcols]
                    )

            def norm(ch):
                nc.vector.reciprocal_approx_fast(recf[ch][:], r_sb[ch][:])
                nc.vector.tensor_copy(rec[ch][:], recf[ch][:])
                for p in range(NP):
                    rb = pp.tile([P, TCH], F32, tag="pp", name=f"rb{ch}_{p}")
                    nc.tensor.matmul(
                        rb[:], esel_sb[:, p * P:(p + 1) * P], rec[ch][:],
                        start=True, stop=True,
                    )
                    nc.vector.tensor_mul(yT[ch][p][:], yT[ch][p][:], rb[:])

            def outproj(ch):
                for mm in range(4):
                    m = ch * 4 + mm
                    for cc in range(2):
                        ps = pp.tile([P, 512], F32, tag="pp", name=f"o_ps{m}_{cc}")
                        for k in range(KC):
                            nc.tensor.matmul(
                                ps[:], yT[ch][k][:, mm * P:(mm + 1) * P],
                                wp(k)[:, c512(cc)],
                                start=(k == 0), stop=(k == KC - 1),
                            )
                        ost = ostp.tile([P, 512], F32, tag="ost", name=f"ost{m}_{cc}")
                        if cc == 0:
                            nc.scalar.copy(ost[:], ps[:])
                        else:
                            nc.vector.tensor_copy(ost[:], ps[:])
                        nc.sync.dma_start(out[m * P:(m + 1) * P, c512(cc)], ost[:])

            kproj_done = None  # emission order below drives scheduler priority
            qproj(0)
            vproj()
            attention(0)
            qproj(1)
            norm(0)
            attention(1)
            outproj(0)
            norm(1)
            outproj(1)

    nc.finalize()
    return nc


def _host_prep_fast(x, feature, Wq, Wf, Wp):
    import ml_dtypes

    bf = ml_dtypes.bfloat16
    f32 = np.float32

    def pack_colblocks(W):
        # pack[p, m*C + k*128 + c] = W.T[k*128+p, m*128+c]
        WT = np.ascontiguousarray(np.asarray(W, f32).T)
        return np.ascontiguousarray(
            WT.reshape(KC, P, KC, P).transpose(1, 2, 0, 3).reshape(P, KC * C).astype(bf)
        )

    def pack_rows(M2d, X):
        # pack[p, k*X + t] = M2d[k*128+p, t]
        return np.ascontiguousarray(
            np.asarray(M2d, f32).reshape(KC, P, X).transpose(1, 0, 2)
            .reshape(P, KC * X).astype(bf)
        )

    i = np.arange(P)[:, None]
    j = np.arange(P)[None, :]
    m0 = (j >= i).astype(f32)
    j2 = np.arange(2 * P)[None, :]
    m1 = (j2 >= (P + i)).astype(f32)

    hsel = np.empty((H,), np.int64)
    hsel[:NP] = 2 * np.arange(NP) + 1
    hsel[NP:] = 2 * np.arange(NP)
    col = np.arange(C)[None, :]
    esel = (hsel[:, None] == col // D).astype(f32)

    shared = {
        "wqp": pack_colblocks(Wq),
        "wkp": pack_colblocks(Wf[:C]),
        "wvp": pack_rows(np.asarray(Wf[C:], f32).T, C),
        "wpp": pack_rows(np.asarray(Wp, f32).T, C),
        "mask0": np.ascontiguousarray(
            np.broadcast_to(m0[:, None, :], (P, 2, P)).astype(bf)
        ),
        "mask1": np.ascontiguousarray(
            np.broadcast_to(m1[:, None, :], (P, 2, 2 * P)).astype(bf)
        ),
        "onesb": np.ones((P, NP), bf),
        "eselb": np.ascontiguousarray(esel.astype(bf)),
    }
    in_maps = []
    for b in range(B):
        m = dict(shared)
        m["xp"] = pack_rows(np.asarray(x[b], f32).T, T)
        m["fp"] = pack_rows(np.asarray(feature[b], f32).T, S)
        in_maps.append(m)
    return in_maps


# --------------------------------------------------------------------------
# slow path (original f32r kernel; used only when LoRA B / bias are nonzero)
# --------------------------------------------------------------------------

def _build_slow(flags):
    has_lq, has_lf, has_lp, has_bq, has_bfk, has_bfv, has_bp = flags
    nc = bacc.Bacc("TRN2", target_bir_lowering=False, debug=False)

    xT = nc.declare_dram_parameter("xT", [C, T], F32R, isOutput=False)
    fT = nc.declare_dram_parameter("fT", [C, S], F32R, isOutput=False)
    WqT = nc.declare_dram_parameter("WqT", [C, C], F32R, isOutput=False)
    WfkT = nc.declare_dram_parameter("WfkT", [C, C], F32R, isOutput=False)
    WfvT = nc.declare_dram_parameter("WfvT", [C, C], F32R, isOutput=False)
    WpT = nc.declare_dram_parameter("WpT", [C, C], F32R, isOutput=False)
    mask = nc.declare_dram_parameter("mask", [P, 384], F32R, isOutput=False)
    vones = nc.declare_dram_parameter("vones", [P, H], F32R, isOutput=False)
    Esel = nc.declare_dram_parameter("Esel", [H, C], F32R, isOutput=False)
    if has_lq:
        AqT = nc.declare_dram_parameter("AqT", [C, R], F32R, isOutput=False)
        BqTs = nc.declare_dram_parameter("BqTs", [R, C], F32R, isOutput=False)
    if has_lf:
        AfT = nc.declare_dram_parameter("AfT", [C, R], F32R, isOutput=False)
        BfkTs = nc.declare_dram_parameter("BfkTs", [R, C], F32R, isOutput=False)
        BfvTs = nc.declare_dram_parameter("BfvTs", [R, C], F32R, isOutput=False)
    if has_lp:
        ApT = nc.declare_dram_parameter("ApT", [C, R], F32R, isOutput=False)
        BpTs = nc.declare_dram_parameter("BpTs", [R, C], F32R, isOutput=False)
    if has_bq:
        bq_pp = nc.declare_dram_parameter("bq_pp", [P, KC], F32, isOutput=False)
    if has_bfk:
        bfk_pp = nc.declare_dram_parameter("bfk_pp", [P, KC], F32, isOutput=False)
    if has_bfv:
        bfv_row = nc.declare_dram_parameter("bfv_row", [1, C], F32R, isOutput=False)
    if has_bp:
        bp_row = nc.declare_dram_parameter("bp_row", [1, C], F32R, isOutput=False)
    out = nc.declare_dram_parameter("out", [T, C], F32, isOutput=True)

    xT3 = xT.rearrange("(ko p) t -> ko p t", p=P)
    fT3 = fT.rearrange("(ko p) s -> ko p s", p=P)
    WqT3 = WqT.rearrange("(ko p) c -> p ko c", p=P)
    WfkT3 = WfkT.rearrange("(ko p) c -> p ko c", p=P)
    WfvT3 = WfvT.rearrange("(ko p) c -> ko p c", p=P)
    WpT3 = WpT.rearrange("(ko p) c -> ko p c", p=P)

    def c512(i):
        return slice(i * 512, (i + 1) * 512)

    with tile.TileContext(nc) as tc:
        with (
            tc.tile_pool(name="big", bufs=8) as big,
            tc.tile_pool(name="qpool", bufs=8) as qpool,
            tc.tile_pool(name="small", bufs=1) as small,
            tc.tile_pool(name="wcol", bufs=3) as wcol,
            tc.tile_pool(name="wrow", bufs=8) as wrow,
            tc.tile_pool(name="expp", bufs=6) as expp,
            tc.tile_pool(name="stg", bufs=3) as stg,
            tc.tile_pool(name="psA", bufs=4, space="PSUM") as psA,
            tc.tile_pool(name="psB", bufs=2, space="PSUM") as psB,
        ):
            xTs = [big.tile([P, T], F32R, tag="big", name=f"xT{k}") for k in range(KC)]
            for k in range(KC):
                nc.sync.dma_start(xTs[k][:], xT3[k])
            fTs = [small.tile([P, S], F32R, tag=f"fT{k}", name=f"fT{k}") for k in range(KC)]
            for k in range(KC):
                nc.sync.dma_start(fTs[k][:], fT3[k])
            mask_sb = small.tile([P, 384], F32R, tag="mask", name="mask_sb")
            nc.sync.dma_start(mask_sb[:], mask[:, :])
            esel_sb = small.tile([H, C], F32R, tag="esel", name="esel_sb")
            nc.sync.dma_start(esel_sb[:], Esel[:, :])
            if has_lq:
                aq_sb = small.tile([P, KC, R], F32R, tag="aq", name="aq_sb")
                nc.sync.dma_start(aq_sb[:], AqT.rearrange("(ko p) r -> p ko r", p=P))
                bqs_sb = small.tile([R, C], F32R, tag="bqs", name="bqs_sb")
                nc.sync.dma_start(bqs_sb[:], BqTs[:, :])
            if has_lf:
                af_sb = small.tile([P, KC, R], F32R, tag="af", name="af_sb")
                nc.sync.dma_start(af_sb[:], AfT.rearrange("(ko p) r -> p ko r", p=P))
                bfks_sb = small.tile([R, C], F32R, tag="bfks", name="bfks_sb")
                nc.sync.dma_start(bfks_sb[:], BfkTs[:, :])
                bfvs_sb = small.tile([R, C], F32R, tag="bfvs", name="bfvs_sb")
                nc.sync.dma_start(bfvs_sb[:], BfvTs[:, :])
            if has_lp:
                ap_sb = small.tile([P, KC, R], F32R, tag="ap", name="ap_sb")
                nc.sync.dma_start(ap_sb[:], ApT.rearrange("(ko p) r -> p ko r", p=P))
                bps_sb = small.tile([R, C], F32R, tag="bps", name="bps_sb")
                nc.sync.dma_start(bps_sb[:], BpTs[:, :])
            if has_bq:
                bq_sb = small.tile([P, KC], F32, tag="bq", name="bq_sb")
                nc.sync.dma_start(bq_sb[:], bq_pp[:, :])
            if has_bfk:
                bfk_sb = small.tile([P, KC], F32, tag="bfk", name="bfk_sb")
                nc.sync.dma_start(bfk_sb[:], bfk_pp[:, :])
            if has_bfv or has_bp:
                ones1 = small.tile([1, P], F32R, tag="ones1", name="ones1")
                nc.sync.dma_start(ones1[:], vones.rearrange("p h -> (p h)")[None, 0:P])
            if has_bfv:
                bfv_sb = small.tile([1, C], F32R, tag="bfv", name="bfv_sb")
                nc.sync.dma_start(bfv_sb[:], bfv_row[:, :])
            if has_bp:
                bp_sb = small.tile([1, C], F32R, tag="bp", name="bp_sb")
                nc.sync.dma_start(bp_sb[:], bp_row[:, :])

            if has_lq:
                ups = psB.tile([P, T], F32, tag="y", name="uq_ps")
                for ch in range(NCH):
                    for k in range(KC):
                        nc.tensor.matmul(
                            ups[:R, c512(ch)], aq_sb[:, k, :], xTs[k][:, c512(ch)],
                            start=(k == 0), stop=(k == KC - 1),
                        )
                uq_sb = small.tile([R, T], F32R, tag="uq", name="uq_sb")
                nc.scalar.copy(uq_sb[:], ups[:R, :])
            if has_lf:
                ufs = psB.tile([P, T], F32, tag="y", name="uf_ps")
                for k in range(KC):
                    nc.tensor.matmul(
                        ufs[:R, :S], af_sb[:, k, :], fTs[k][:],
                        start=(k == 0), stop=(k == KC - 1),
                    )
                uf_sb = small.tile([R, S], F32R, tag="uf", name="uf_sb")
                nc.scalar.copy(uf_sb[:], ufs[:R, :S])

            kTs = [small.tile([P, S], F32R, tag=f"kT{m}", name=f"kT{m}") for m in range(KC)]
            for m in range(KC):
                wk_m = wcol.tile([P, KC, P], F32R, tag="wcol", name=f"wk{m}")
                nc.sync.dma_start(wk_m[:], WfkT3[:, :, m * P:(m + 1) * P])
                ps = psA.tile([P, S], F32, tag="mm", name=f"k_ps{m}")
                for k in range(KC):
                    nc.tensor.matmul(
                        ps[:], wk_m[:, k, :], fTs[k][:],
                        start=(k == 0), stop=(k == KC - 1 and not has_lf),
                    )
                if has_lf:
                    nc.tensor.matmul(
                        ps[:], bfks_sb[:, m * P:(m + 1) * P], uf_sb[:],
                        start=False, stop=True,
                    )
                if has_bfk:
                    nc.scalar.activation(
                        kTs[m][:], ps[:], mybir.ActivationFunctionType.Identity,
                        bias=bfk_sb[:, m:m + 1], scale=1.0,
                    )
                else:
                    nc.vector.tensor_copy(kTs[m][:], ps[:])

            wfv = [wrow.tile([P, C], F32R, tag="wrow", name=f"wfv{k}") for k in range(KC)]
            for k in range(KC):
                nc.sync.dma_start(wfv[k][:], WfvT3[k])
            v_aug = [
                small.tile([P, H, D + 1], F32R, tag=f"vaug{s2}", name=f"vaug{s2}")
                for s2 in range(2)
            ]
            for s2 in range(2):
                nc.sync.dma_start(v_aug[s2][:, :, D], vones[:, :])
                for ch in range(NCH):
                    ps = psA.tile([P, 512], F32, tag="mm", name=f"v_ps{s2}_{ch}")
                    nmm = KC + (1 if has_lf else 0) + (1 if has_bfv else 0)
                    i = 0
                    for k in range(KC):
                        i += 1
                        nc.tensor.matmul(
                            ps[:], fTs[k][:, s2 * P:(s2 + 1) * P],
                            wfv[k][:, c512(ch)],
                            start=(i == 1), stop=(i == nmm),
                        )
                    if has_lf:
                        i += 1
                        nc.tensor.matmul(
                            ps[:], uf_sb[:, s2 * P:(s2 + 1) * P],
                            bfvs_sb[:, c512(ch)], start=False, stop=(i == nmm),
                        )
                    if has_bfv:
                        i += 1
                        nc.tensor.matmul(
                            ps[:], ones1[:], bfv_sb[:, c512(ch)],
                            start=False, stop=(i == nmm),
                        )
                    for hh in range(8):
                        h = ch * 8 + hh
                        nc.vector.tensor_copy(
                            v_aug[s2][:, h, 0:D], ps[:, hh * D:(hh + 1) * D]
                        )

            qTs = [qpool.tile([P, T], F32R, tag="qT", name=f"qT{m}") for m in range(MT)]
            for m in range(KC):
                wq_m = wcol.tile([P, KC, P], F32R, tag="wcol", name=f"wq{m}")
                nc.sync.dma_start(wq_m[:], WqT3[:, :, m * P:(m + 1) * P])
                for ch in range(NCH):
                    ps = psA.tile([P, 512], F32, tag="mm", name=f"q_ps{m}_{ch}")
                    for k in range(KC):
                        nc.tensor.matmul(
                            ps[:], wq_m[:, k, :], xTs[k][:, c512(ch)],
                            start=(k == 0), stop=(k == KC - 1 and not has_lq),
                        )
                    if has_lq:
                        nc.tensor.matmul(
                            ps[:], bqs_sb[:, m * P:(m + 1) * P], uq_sb[:, c512(ch)],
                            start=False, stop=True,
                        )
                    if has_bq:
                        nc.scalar.activation(
                            qTs[m][:, c512(ch)], ps[:],
                            mybir.ActivationFunctionType.Identity,
                            bias=bq_sb[:, m:m + 1], scale=1.0,
                        )
                    else:
                        nc.vector.tensor_copy(qTs[m][:, c512(ch)], ps[:])

            yTr = [big.tile([P, T], F32R, tag="big", name=f"yTr{p}") for p in range(KC)]
            r_sb = small.tile([H, T], F32R, tag="rsum", name="r_sb")
            for h in range(H):
                m, off = h // 2, (h % 2) * D
                kt_h = kTs[m][off:off + D, :]
                qt_h = qTs[m][off:off + D, :]
                es = [expp.tile([P, T], F32R, tag="exp", name=f"e{h}_{s2}") for s2 in range(2)]
                for s2 in range(2):
                    for ch in range(NCH):
                        ps = psA.tile([P, 512], F32, tag="mm", name=f"s_ps{h}_{s2}_{ch}")
                        nc.tensor.matmul(
                            ps[:], kt_h[:, s2 * P:(s2 + 1) * P], qt_h[:, c512(ch)],
                            start=True, stop=True,
                        )
                        nc.scalar.activation(
                            es[s2][:, c512(ch)], ps[:],
                            mybir.ActivationFunctionType.Exp, scale=0.125,
                        )
                nc.vector.tensor_mul(es[0][:, 0:P], es[0][:, 0:P], mask_sb[:, 0:P])
                nc.vector.tensor_mul(es[1][:, 0:S], es[1][:, 0:S], mask_sb[:, P:384])
                psy = psB.tile([P, T], F32, tag="y", name=f"y_ps{h}")
                for ch in range(NCH):
                    for s2 in range(2):
                        nc.tensor.matmul(
                            psy[:D + 1, c512(ch)], v_aug[s2][:, h, :],
                            es[s2][:, c512(ch)], start=(s2 == 0), stop=(s2 == 1),
                        )
                st = stg.tile([P, T], F32R, tag="hstage", name=f"st{h}")
                if off == 0:
                    nc.vector.tensor_copy(yTr[m][0:D, :], psy[0:D, :])
                    nc.vector.tensor_copy(st[D:D + 1, :], psy[D:D + 1, :])
                else:
                    nc.vector.tensor_copy(st[0:D + 1, :], psy[0:D + 1, :])
                    nc.sync.dma_start(yTr[m][off:off + D, :], st[0:D, :])
                nc.sync.dma_start(r_sb[h:h + 1, :], st[D:D + 1, :])

            recf = small.tile([H, T], F32, tag="recf", name="recf")
            nc.vector.reciprocal(recf[:], r_sb[:])
            rec = small.tile([H, T], F32R, tag="rec", name="rec")
            nc.vector.tensor_copy(rec[:], recf[:])
            for p in range(KC):
                rb = psB.tile([P, T], F32, tag="y", name=f"rb{p}")
                for ch in range(NCH):
                    nc.tensor.matmul(
                        rb[:, c512(ch)], esel_sb[:, p * P:(p + 1) * P],
                        rec[:, c512(ch)], start=True, stop=True,
                    )
                nc.vector.tensor_mul(yTr[p][:], yTr[p][:], rb[:])

            if has_lp:
                upsd = psB.tile([P, T], F32, tag="y", name="up_ps")
                for ch in range(NCH):
                    for k in range(KC):
                        nc.tensor.matmul(
                            upsd[:R, c512(ch)], ap_sb[:, k, :], yTr[k][:, c512(ch)],
                            start=(k == 0), stop=(k == KC - 1),
                        )
                up_sb = small.tile([R, T], F32R, tag="up", name="up_sb")
                nc.scalar.copy(up_sb[:], upsd[:R, :])
            wp = [wrow.tile([P, C], F32R, tag="wrow", name=f"wp{k}") for k in range(KC)]
            for k in range(KC):
                nc.sync.dma_start(wp[k][:], WpT3[k])
            for m in range(MT):
                for ch in range(NCH):
                    ps = psA.tile([P, 512], F32, tag="mm", name=f"o_ps{m}_{ch}")
                    nmm = KC + (1 if has_lp else 0) + (1 if has_bp else 0)
                    i = 0
                    for k in range(KC):
                        i += 1
                        nc.tensor.matmul(
                            ps[:], yTr[k][:, m * P:(m + 1) * P], wp[k][:, c512(ch)],
                            start=(i == 1), stop=(i == nmm),
                        )
                    if has_lp:
                        i += 1
                        nc.tensor.matmul(
                            ps[:], up_sb[:, m * P:(m + 1) * P], bps_sb[:, c512(ch)],
                            start=False, stop=(i == nmm),
                        )
                    if has_bp:
                        i += 1
                        nc.tensor.matmul(
                            ps[:], ones1[:], bp_sb[:, c512(ch)],
                            start=False, stop=(i == nmm),
                        )
                    ost = wcol.tile([P, 512], F32, tag="ostage", name=f"ost{m}_{ch}")
                    nc.scalar.copy(ost[:], ps[:])
                    nc.sync.dma_start(out[m * P:(m + 1) * P, c512(ch)], ost[:])

    nc.finalize()
    return nc


def _host_prep_slow(x, feature, Wq, bq, Aq, Bq, Wf, bf, Af, Bf, Wp, bp, Ap, Bp, flags):
    f32 = np.float32
    shared = {
        "WqT": np.ascontiguousarray(np.asarray(Wq, f32).T),
        "WfkT": np.ascontiguousarray(np.asarray(Wf[:C], f32).T),
        "WfvT": np.ascontiguousarray(np.asarray(Wf[C:], f32).T),
        "WpT": np.ascontiguousarray(np.asarray(Wp, f32).T),
    }
    i = np.arange(P)[:, None]
    j = np.arange(384)[None, :]
    m0 = (j[:, :P] >= i).astype(f32)
    m1 = ((j[:, P:384] - P) >= (P + i)).astype(f32)
    shared["mask"] = np.ascontiguousarray(np.concatenate([m0, m1], axis=1))
    shared["vones"] = np.ones((P, H), f32)
    hsel = np.arange(H)[:, None]
    col = np.arange(C)[None, :]
    shared["Esel"] = np.ascontiguousarray((hsel == col // D).astype(f32))
    has_lq, has_lf, has_lp, has_bq, has_bfk, has_bfv, has_bp = flags
    if has_lq:
        shared["AqT"] = np.ascontiguousarray(np.asarray(Aq, f32).T)
        shared["BqTs"] = np.ascontiguousarray(np.asarray(Bq, f32).T * SCALING)
    if has_lf:
        shared["AfT"] = np.ascontiguousarray(np.asarray(Af, f32).T)
        shared["BfkTs"] = np.ascontiguousarray(np.asarray(Bf[:C], f32).T * SCALING)
        shared["BfvTs"] = np.ascontiguousarray(np.asarray(Bf[C:], f32).T * SCALING)
    if has_lp:
        shared["ApT"] = np.ascontiguousarray(np.asarray(Ap, f32).T)
        shared["BpTs"] = np.ascontiguousarray(np.asarray(Bp, f32).T * SCALING)
    if has_bq:
        shared["bq_pp"] = np.ascontiguousarray(np.asarray(bq, f32).reshape(KC, P).T)
    if has_bfk:
        shared["bfk_pp"] = np.ascontiguousarray(np.asarray(bf[:C], f32).reshape(KC, P).T)
    if has_bfv:
        shared["bfv_row"] = np.ascontiguousarray(np.asarray(bf[C:], f32).reshape(1, C))
    if has_bp:
        shared["bp_row"] = np.ascontiguousarray(np.asarray(bp, f32).reshape(1, C))

    in_maps = []
    for b in range(B):
        m = dict(shared)
        m["xT"] = np.ascontiguousarray(np.asarray(x[b], f32).T)
        m["fT"] = np.ascontiguousarray(np.asarray(feature[b], f32).T)
        in_maps.append(m)
    return in_maps


# --------------------------------------------------------------------------
# dispatch
# --------------------------------------------------------------------------

def _run(inputs, trace=False, **spmd_kwargs):
    x, feature = inputs["x"], inputs["feature"]
    Wq, bq, Aq, Bq = inputs["Wq"], inputs["bq"], inputs["Aq"], inputs["Bq"]
    Wf, bf_, Af, Bf = inputs["Wf"], inputs["bf"], inputs["Af"], inputs["Bf"]
    Wp, bp, Ap, Bp = inputs["Wp"], inputs["bp"], inputs["Ap"], inputs["Bp"]
    flags = (
        bool(np.any(Bq)), bool(np.any(Bf)), bool(np.any(Bp)),
        bool(np.any(bq)), bool(np.any(bf_[:C])), bool(np.any(bf_[C:])),
        bool(np.any(bp)),
    )
    if any(flags):
        key = ("slow", flags)
        nc = _nc_cache.get(key)
        if nc is None:
            nc = _build_slow(flags)
            _nc_cache[key] = nc
        in_maps = _host_prep_slow(
            x, feature, Wq, bq, Aq, Bq, Wf, bf_, Af, Bf, Wp, bp, Ap, Bp, flags
        )
    else:
        key = "fast"
        nc = _nc_cache.get(key)
        if nc is None:
            nc = _build_fast()
            _nc_cache[key] = nc
        in_maps = _host_prep_fast(x, feature, Wq, Wf, Wp)
    res = run_bass_kernel_spmd(
        nc, in_maps, core_ids=list(range(B)), trace=trace, **spmd_kwargs
    )
    out = np.stack([res.results[b]["out"] for b in range(B)], axis=0)
    return out, res


def kernel(**inputs):
    out, _ = _run(inputs, trace=False)
    return out


# revision 15
# speedup vs baseline: 1.2034x; 1.0028x over previous
"""Trainium2 Bass kernel for CrossAttention with LoRA.

Data-parallel over batch (B=8 -> 8 NeuronCores, one batch element per core).
No collectives.

Fast path (the actual case: loralib-init B matrices and biases are all zero,
so LoRA/bias terms vanish): fully fused bf16 cross-attention with a T-chunked
pipeline -- see _build_fast. Key tricks:
  * all matmul operands bf16 (halves DMA/SBUF; PE cost unchanged; psum f32)
  * score matmuls for an even/odd head pair issued at PE row-tile positions
    (0,0)/(64,0) -- K=64 each, run concurrently in the PE array
  * attn@v uses a zero-padded M=128 stationary per head:
      even head [v(64) | 1 | 0...]: y -> psum rows 0:64, denom -> row 64
      odd  head [0... | 1 | v(64)]: denom -> row 63, y -> rows 64:128
    so every psum drain is partition-aligned (no SBUF->SBUF bounce)
  * attention is chunked over T (2 x 512): chunk 0's softmax-normalize and
    output projection overlap chunk 1's attention; q-projection of chunk 1
    fills PE gaps during chunk 0's attention (Tile's list scheduler pops
    ready work in emission order)
  * weights DMA'd in consumption order, packed contiguously per-tile on host

Slow path (any nonzero LoRA B / bias): the original f32r kernel, kept
verbatim for correctness insurance.
"""

import numpy as np

import concourse.bass as bass  # noqa: F401  (bass types via bacc)
import concourse.mybir as mybir
import concourse.tile as tile
from concourse import bacc
from concourse.bass_utils import run_bass_kernel_spmd

B, T, S, C, H, D, R = 8, 1024, 256, 1024, 16, 64, 16
SCALING = 1.0 / 16.0
P = 128
KC = C // P  # 8 k-tiles over the embedding dim
MT = T // P  # 8 tiles over T
NP = H // 2  # 8 head pairs
NCH = 2      # t-chunks (fast path) / psum chunks (slow path)
TCH = T // NCH
F32 = mybir.dt.float32
F32R = mybir.dt.float32r
BF16 = mybir.dt.bfloat16

_nc_cache: dict = {}


# --------------------------------------------------------------------------
# fast path
# --------------------------------------------------------------------------

def _build_fast():
    nc = bacc.Bacc("TRN2", target_bir_lowering=False, debug=False)

    # host-packed contiguous per-partition layouts:
    #   xp{ch}[p, k*TCH + t] = x.T[k*128+p, ch*TCH+t]
    #   fp[p, k*S + s]       = f.T[k*128+p, s]
    #   wqp/wkp[p, m*C + k*128 + c] = W.T[k*128+p, m*128+c]   (m-major)
    #   wvp/wpp[p, k*C + c]  = W.T[k*128+p, c]                (k-major)
    xp = nc.declare_dram_parameter("xp", [P, KC * T], BF16, isOutput=False)
    fp = nc.declare_dram_parameter("fp", [P, KC * S], BF16, isOutput=False)
    wqp = nc.declare_dram_parameter("wqp", [P, KC * C], BF16, isOutput=False)
    wkp = nc.declare_dram_parameter("wkp", [P, KC * C], BF16, isOutput=False)
    wvp = nc.declare_dram_parameter("wvp", [P, KC * C], BF16, isOutput=False)
    wpp = nc.declare_dram_parameter("wpp", [P, KC * C], BF16, isOutput=False)
    mask0 = nc.declare_dram_parameter("mask0", [P, 2, P], BF16, isOutput=False)
    mask1 = nc.declare_dram_parameter("mask1", [P, 2, 2 * P], BF16, isOutput=False)
    onesb = nc.declare_dram_parameter("onesb", [P, NP], BF16, isOutput=False)
    eselb = nc.declare_dram_parameter("eselb", [H, C], BF16, isOutput=False)
    out = nc.declare_dram_parameter("out", [T, C], F32, isOutput=True)

    HC = 4 * C

    def c512(i):
        return slice(i * 512, (i + 1) * 512)

    with tile.TileContext(nc) as tc:
        with (
            tc.tile_pool(name="res", bufs=1) as res,
            tc.tile_pool(name="espool", bufs=6) as espool,
            tc.tile_pool(name="ostp", bufs=3) as ostp,
            tc.tile_pool(name="pp", bufs=2, space="PSUM") as pp,
            tc.tile_pool(name="psS", bufs=2, space="PSUM") as psS,
            tc.tile_pool(name="psy", bufs=2, space="PSUM") as psy,
        ):
            # ---- resident loads: one sync-queue stream in consumption order
            # (per-queue FIFO serializes the transfers, so arrival order ==
            # issue order and nothing competes with the critical early loads)
            # per-piece 2KB/partition tiles; one serialized sync-queue
            # stream in consumption order (per-queue FIFO => arrival order)
            fTs = [res.tile([P, S], BF16, tag=f"fT{k}", name=f"fT{k}") for k in range(KC)]
            wks = [res.tile([P, C], BF16, tag=f"wk{m}", name=f"wk{m}") for m in range(KC)]
            wqs = [res.tile([P, C], BF16, tag=f"wq{m}", name=f"wq{m}") for m in range(KC)]
            xts = [
                [res.tile([P, TCH], BF16, tag=f"xt{ch}_{k}", name=f"xt{ch}_{k}")
                 for k in range(KC)]
                for ch in range(NCH)
            ]
            wvs = [res.tile([P, C], BF16, tag=f"wv{k}", name=f"wv{k}") for k in range(KC)]
            wps = [res.tile([P, C], BF16, tag=f"wp{k}", name=f"wp{k}") for k in range(KC)]
            for k in range(KC):
                nc.sync.dma_start(fTs[k][:], fp[:, k * S:(k + 1) * S])
            for m in range(KC):
                nc.sync.dma_start(wks[m][:], wkp[:, m * C:(m + 1) * C])
            for m in range(KC):
                nc.sync.dma_start(wqs[m][:], wqp[:, m * C:(m + 1) * C])
                nc.sync.dma_start(
                    xts[0][m][:], xp[:, m * T:m * T + TCH]
                )
            m0_sb = res.tile([P, 2, P], BF16, tag="m0", name="m0_sb")
            nc.sync.dma_start(m0_sb[:], mask0[:, :, :])
            m1_sb = res.tile([P, 2, 2 * P], BF16, tag="m1", name="m1_sb")
            nc.sync.dma_start(m1_sb[:], mask1[:, :, :])
            esel_sb = res.tile([H, C], BF16, tag="esel", name="esel_sb")
            nc.sync.dma_start(esel_sb[:], eselb[:, :])
            for k in range(KC):
                nc.sync.dma_start(wvs[k][:], wvp[:, k * C:(k + 1) * C])
            for k in range(KC):
                nc.sync.dma_start(
                    xts[1][k][:], xp[:, k * T + TCH:(k + 1) * T]
                )
            for k in range(KC):
                nc.sync.dma_start(wps[k][:], wpp[:, k * C:(k + 1) * C])

            def fT(k):
                return fTs[k][:]

            def wfk(m, k):
                return wks[m][:, k * P:(k + 1) * P]

            def wq(m, k):
                return wqs[m][:, k * P:(k + 1) * P]

            def wfv(k):
                return wvs[k][:]

            def wp(k):
                return wps[k][:]

            def xT(ch, k):
                return xts[ch][k][:]

            # v_aug[s2]: [s(128), pair(8), parity(2), col(128)] bf16
            # even head (parity 0): cols 0:64 = v, col 64 = 1  -> y at psum
            #   rows 0:64, denominator at row 64
            # odd  head (parity 1): col 0 = 1, cols 64:128 = v -> denominator
            #   at psum row 0, y at rows 64:128
            v_aug = [
                res.tile([P, NP, 2, P], BF16, tag=f"vaug{s2}", name=f"vaug{s2}")
                for s2 in range(2)
            ]
            for s2 in range(2):
                nc.vector.memset(v_aug[s2][:], 0.0)
                nc.sync.dma_start(
                    v_aug[s2][:, :, 0:1, 64:65], onesb[:, :, None, None]
                )
                nc.sync.dma_start(
                    v_aug[s2][:, :, 1:2, 0:1], onesb[:, :, None, None]
                )

            # ---- k projection: kT[m] [128, S] ------------------------------
            kTs = [res.tile([P, S], BF16, tag=f"kT{m}", name=f"kT{m}") for m in range(KC)]
            for m in range(KC):
                ps = pp.tile([P, S], F32, tag="pp", name=f"k_ps{m}")
                for k in range(KC):
                    nc.tensor.matmul(
                        ps[:], wfk(m, k), fT(k),
                        start=(k == 0), stop=(k == KC - 1),
                    )
                nc.vector.tensor_copy(kTs[m][:], ps[:])

            qT = [
                [res.tile([P, TCH], BF16, tag=f"qT{ch}_{m}", name=f"qT{ch}_{m}")
                 for m in range(KC)]
                for ch in range(NCH)
            ]
            yT = [
                [res.tile([P, TCH], BF16, tag=f"yT{ch}_{p}", name=f"yT{ch}_{p}")
                 for p in range(NP)]
                for ch in range(NCH)
            ]
            rstage = [
                res.tile([P, NP * TCH], F32, tag=f"rstg{ch}", name=f"rstg{ch}")
                for ch in range(NCH)
            ]
            r_sb = [
                res.tile([H, TCH], F32, tag=f"rsb{ch}", name=f"rsb{ch}")
                for ch in range(NCH)
            ]
            recf = [
                res.tile([H, TCH], F32, tag=f"recf{ch}", name=f"recf{ch}")
                for ch in range(NCH)
            ]
            rec = [
                res.tile([H, TCH], BF16, tag=f"rec{ch}", name=f"rec{ch}")
                for ch in range(NCH)
            ]

            def qproj(ch):
                for m in range(KC):
                    ps = pp.tile([P, TCH], F32, tag="pp", name=f"q_ps{ch}_{m}")
                    for k in range(KC):
                        nc.tensor.matmul(
                            ps[:], wq(m, k), xT(ch, k),
                            start=(k == 0), stop=(k == KC - 1),
                        )
                    nc.vector.tensor_copy(qT[ch][m][:], ps[:])

            def vproj():
                for s2 in range(2):
                    for cc in range(2):
                        ps = pp.tile([P, 4, 2, D], F32, tag="pp", name=f"v_ps{s2}_{cc}")
                        for k in range(KC):
                            nc.tensor.matmul(
                                ps[:], fT(k)[:, s2 * P:(s2 + 1) * P],
                                wfv(k)[:, c512(cc)],
                                start=(k == 0), stop=(k == KC - 1),
                            )
                        nc.vector.tensor_copy(
                            v_aug[s2][:, cc * 4:(cc + 1) * 4, 0:1, 0:D],
                            ps[:, :, 0:1, :],
                        )
                        nc.vector.tensor_copy(
                            v_aug[s2][:, cc * 4:(cc + 1) * 4, 1:2, D:2 * D],
                            ps[:, :, 1:2, :],
                        )

            def attention(ch):
                for p in range(NP):
                    es = [
                        espool.tile([P, 2, TCH], BF16, tag="es", name=f"es{ch}_{p}_{s2}")
                        for s2 in range(2)
                    ]
                    for s2 in range(2):
                        ps = psS.tile(
                            [P, 2 * TCH], F32, tag="psS", name=f"s_ps{ch}_{p}_{s2}"
                        )
                        nc.tensor.matmul(
                            ps[:, 0:TCH],
                            kTs[p][0:D, s2 * P:(s2 + 1) * P],
                            qT[ch][p][0:D, :],
                            start=True, stop=True,
                        )
                        nc.tensor.matmul(
                            ps[:, TCH:2 * TCH],
                            kTs[p][D:P, s2 * P:(s2 + 1) * P],
                            qT[ch][p][D:P, :],
                            start=True, stop=True,
                        )
                        nc.scalar.activation(
                            es[s2][:], ps[:],
                            mybir.ActivationFunctionType.Exp, scale=0.125,
                        )
                    if ch == 0:
                        nc.gpsimd.tensor_mul(
                            es[0][:, :, 0:P], es[0][:, :, 0:P], m0_sb[:]
                        )
                        nc.gpsimd.tensor_mul(
                            es[1][:, :, 0:2 * P], es[1][:, :, 0:2 * P], m1_sb[:]
                        )
                    psa = psy.tile([P, TCH], F32, tag="psy", name=f"ya{ch}_{p}")
                    psb = psy.tile([P, TCH], F32, tag="psy", name=f"yb{ch}_{p}")
                    for s2 in range(2):
                        nc.tensor.matmul(
                            psa[:], v_aug[s2][:, p:p + 1, 0:1, :],
                            es[s2][:, 0:1, :],
                            start=(s2 == 0), stop=(s2 == 1),
                        )
                    for s2 in range(2):
                        nc.tensor.matmul(
                            psb[:], v_aug[s2][:, p:p + 1, 1:2, :],
                            es[s2][:, 1:2, :],
                            start=(s2 == 0), stop=(s2 == 1),
                        )
                    pcols = slice(p * TCH, (p + 1) * TCH)
                    nc.vector.tensor_copy(yT[ch][p][0:D, :], psa[0:D, :])
                    nc.scalar.copy(rstage[ch][64:65, pcols], psa[64:65, :])
                    nc.vector.tensor_copy(yT[ch][p][D:P, :], psb[D:P, :])
                    nc.vector.tensor_copy(rstage[ch][0:1, pcols], psb[0:1, :])
                    # gather denom rows: r_sb rows 0:8 = odd heads, 8:16 = even
                    nc.sync.dma_start(
                        r_sb[ch][8 + p:9 + p, :], rstage[ch][64:65, pcols]
                    )
                    nc.sync.dma_start(
                        r_sb[ch][p:p + 1, :], rstage[ch][0:1, pcols]
                    )

            def norm(ch):
                nc.vector.reciprocal_approx_fast(recf[ch][:], r_sb[ch][:])
                nc.vector.tensor_copy(rec[ch][:], recf[ch][:])
                for p in range(NP):
                    rb = pp.tile([P, TCH], F32, tag="pp", name=f"rb{ch}_{p}")
                    nc.tensor.matmul(
                        rb[:], esel_sb[:, p * P:(p + 1) * P], rec[ch][:],
                        start=True, stop=True,
                    )
                    nc.vector.tensor_mul(yT[ch][p][:], yT[ch][p][:], rb[:])

            def outproj(ch):
                for mm in range(4):
                    m = ch * 4 + mm
                    for cc in range(2):
                        ps = pp.tile([P, 512], F32, tag="pp", name=f"o_ps{m}_{cc}")
                        for k in range(KC):
                            nc.tensor.matmul(
                                ps[:], yT[ch][k][:, mm * P:(mm + 1) * P],
                                wp(k)[:, c512(cc)],
                                start=(k == 0), stop=(k == KC - 1),
                            )
                        ost = ostp.tile([P, 512], F32, tag="ost", name=f"ost{m}_{cc}")
                        nc.scalar.copy(ost[:], ps[:])
                        nc.sync.dma_start(out[m * P:(m + 1) * P, c512(cc)], ost[:])

            kproj_done = None  # emission order below drives scheduler priority
            qproj(0)
            vproj()
            attention(0)
            qproj(1)
            norm(0)
            attention(1)
            outproj(0)
            norm(1)
            outproj(1)

    nc.finalize()
    return nc


def _host_prep_fast(x, feature, Wq, Wf, Wp):
    import ml_dtypes

    bf = ml_dtypes.bfloat16
    f32 = np.float32

    def pack_colblocks(W):
        # pack[p, m*C + k*128 + c] = W.T[k*128+p, m*128+c]
        WT = np.ascontiguousarray(np.asarray(W, f32).T)
        return np.ascontiguousarray(
            WT.reshape(KC, P, KC, P).transpose(1, 2, 0, 3).reshape(P, KC * C).astype(bf)
        )

    def pack_rows(M2d, X):
        # pack[p, k*X + t] = M2d[k*128+p, t]
        return np.ascontiguousarray(
            np.asarray(M2d, f32).reshape(KC, P, X).transpose(1, 0, 2)
            .reshape(P, KC * X).astype(bf)
        )

    i = np.arange(P)[:, None]
    j = np.arange(P)[None, :]
    m0 = (j >= i).astype(f32)
    j2 = np.arange(2 * P)[None, :]
    m1 = (j2 >= (P + i)).astype(f32)

    hsel = np.empty((H,), np.int64)
    hsel[:NP] = 2 * np.arange(NP) + 1
    hsel[NP:] = 2 * np.arange(NP)
    col = np.arange(C)[None, :]
    esel = (hsel[:, None] == col // D).astype(f32)

    shared = {
        "wqp": pack_colblocks(Wq),
        "wkp": pack_colblocks(Wf[:C]),
        "wvp": pack_rows(np.asarray(Wf[C:], f32).T, C),
        "wpp": pack_rows(np.asarray(Wp, f32).T, C),
        "mask0": np.ascontiguousarray(
            np.broadcast_to(m0[:, None, :], (P, 2, P)).astype(bf)
        ),
        "mask1": np.ascontiguousarray(
            np.broadcast_to(m1[:, None, :], (P, 2, 2 * P)).astype(bf)
        ),
        "onesb": np.ones((P, NP), bf),
        "eselb": np.ascontiguousarray(esel.astype(bf)),
    }
    in_maps = []
    for b in range(B):
        m = dict(shared)
        m["xp"] = pack_rows(np.asarray(x[b], f32).T, T)
        m["fp"] = pack_rows(np.asarray(feature[b], f32).T, S)
        in_maps.append(m)
    return in_maps


# --------------------------------------------------------------------------
# slow path (original f32r kernel; used only when LoRA B / bias are nonzero)
# --------------------------------------------------------------------------

def _build_slow(flags):
    has_lq, has_lf, has_lp, has_bq, has_bfk, has_bfv, has_bp = flags
    nc = bacc.Bacc("TRN2", target_bir_lowering=False, debug=False)

    xT = nc.declare_dram_parameter("xT", [C, T], F32R, isOutput=False)
    fT = nc.declare_dram_parameter("fT", [C, S], F32R, isOutput=False)
    WqT = nc.declare_dram_parameter("WqT", [C, C], F32R, isOutput=False)
    WfkT = nc.declare_dram_parameter("WfkT", [C, C], F32R, isOutput=False)
    WfvT = nc.declare_dram_parameter("WfvT", [C, C], F32R, isOutput=False)
    WpT = nc.declare_dram_parameter("WpT", [C, C], F32R, isOutput=False)
    mask = nc.declare_dram_parameter("mask", [P, 384], F32R, isOutput=False)
    vones = nc.declare_dram_parameter("vones", [P, H], F32R, isOutput=False)
    Esel = nc.declare_dram_parameter("Esel", [H, C], F32R, isOutput=False)
    if has_lq:
        AqT = nc.declare_dram_parameter("AqT", [C, R], F32R, isOutput=False)
        BqTs = nc.declare_dram_parameter("BqTs", [R, C], F32R, isOutput=False)
    if has_lf:
        AfT = nc.declare_dram_parameter("AfT", [C, R], F32R, isOutput=False)
        BfkTs = nc.declare_dram_parameter("BfkTs", [R, C], F32R, isOutput=False)
        BfvTs = nc.declare_dram_parameter("BfvTs", [R, C], F32R, isOutput=False)
    if has_lp:
        ApT = nc.declare_dram_parameter("ApT", [C, R], F32R, isOutput=False)
        BpTs = nc.declare_dram_parameter("BpTs", [R, C], F32R, isOutput=False)
    if has_bq:
        bq_pp = nc.declare_dram_parameter("bq_pp", [P, KC], F32, isOutput=False)
    if has_bfk:
        bfk_pp = nc.declare_dram_parameter("bfk_pp", [P, KC], F32, isOutput=False)
    if has_bfv:
        bfv_row = nc.declare_dram_parameter("bfv_row", [1, C], F32R, isOutput=False)
    if has_bp:
        bp_row = nc.declare_dram_parameter("bp_row", [1, C], F32R, isOutput=False)
    out = nc.declare_dram_parameter("out", [T, C], F32, isOutput=True)

    xT3 = xT.rearrange("(ko p) t -> ko p t", p=P)
    fT3 = fT.rearrange("(ko p) s -> ko p s", p=P)
    WqT3 = WqT.rearrange("(ko p) c -> p ko c", p=P)
    WfkT3 = WfkT.rearrange("(ko p) c -> p ko c", p=P)
    WfvT3 = WfvT.rearrange("(ko p) c -> ko p c", p=P)
    WpT3 = WpT.rearrange("(ko p) c -> ko p c", p=P)

    def c512(i):
        return slice(i * 512, (i + 1) * 512)

    with tile.TileContext(nc) as tc:
        with (
            tc.tile_pool(name="big", bufs=8) as big,
            tc.tile_pool(name="qpool", bufs=8) as qpool,
            tc.tile_pool(name="small", bufs=1) as small,
            tc.tile_pool(name="wcol", bufs=3) as wcol,
            tc.tile_pool(name="wrow", bufs=8) as wrow,
            tc.tile_pool(name="expp", bufs=6) as expp,
            tc.tile_pool(name="stg", bufs=3) as stg,
            tc.tile_pool(name="psA", bufs=4, space="PSUM") as psA,
            tc.tile_pool(name="psB", bufs=2, space="PSUM") as psB,
        ):
            xTs = [big.tile([P, T], F32R, tag="big", name=f"xT{k}") for k in range(KC)]
            for k in range(KC):
                nc.sync.dma_start(xTs[k][:], xT3[k])
            fTs = [small.tile([P, S], F32R, tag=f"fT{k}", name=f"fT{k}") for k in range(KC)]
            for k in range(KC):
                nc.sync.dma_start(fTs[k][:], fT3[k])
            mask_sb = small.tile([P, 384], F32R, tag="mask", name="mask_sb")
            nc.sync.dma_start(mask_sb[:], mask[:, :])
            esel_sb = small.tile([H, C], F32R, tag="esel", name="esel_sb")
            nc.sync.dma_start(esel_sb[:], Esel[:, :])
            if has_lq:
                aq_sb = small.tile([P, KC, R], F32R, tag="aq", name="aq_sb")
                nc.sync.dma_start(aq_sb[:], AqT.rearrange("(ko p) r -> p ko r", p=P))
                bqs_sb = small.tile([R, C], F32R, tag="bqs", name="bqs_sb")
                nc.sync.dma_start(bqs_sb[:], BqTs[:, :])
            if has_lf:
                af_sb = small.tile([P, KC, R], F32R, tag="af", name="af_sb")
                nc.sync.dma_start(af_sb[:], AfT.rearrange("(ko p) r -> p ko r", p=P))
                bfks_sb = small.tile([R, C], F32R, tag="bfks", name="bfks_sb")
                nc.sync.dma_start(bfks_sb[:], BfkTs[:, :])
                bfvs_sb = small.tile([R, C], F32R, tag="bfvs", name="bfvs_sb")
                nc.sync.dma_start(bfvs_sb[:], BfvTs[:, :])
            if has_lp:
                ap_sb = small.tile([P, KC, R], F32R, tag="ap", name="ap_sb")
                nc.sync.dma_start(ap_sb[:], ApT.rearrange("(ko p) r -> p ko r", p=P))
                bps_sb = small.tile([R, C], F32R, tag="bps", name="bps_sb")
                nc.sync.dma_start(bps_sb[:], BpTs[:, :])
            if has_bq:
                bq_sb = small.tile([P, KC], F32, tag="bq", name="bq_sb")
                nc.sync.dma_start(bq_sb[:], bq_pp[:, :])
            if has_bfk:
                bfk_sb = small.tile([P, KC], F32, tag="bfk", name="bfk_sb")
                nc.sync.dma_start(bfk_sb[:], bfk_pp[:, :])
            if has_bfv or has_bp:
                ones1 = small.tile([1, P], F32R, tag="ones1", name="ones1")
                nc.sync.dma_start(ones1[:], vones.rearrange("p h -> (p h)")[None, 0:P])
            if has_bfv:
                bfv_sb = small.tile([1, C], F32R, tag="bfv", name="bfv_sb")
                nc.sync.dma_start(bfv_sb[:], bfv_row[:, :])
            if has_bp:
                bp_sb = small.tile([1, C], F32R, tag="bp", name="bp_sb")
                nc.sync.dma_start(bp_sb[:], bp_row[:, :])

            if has_lq:
                ups = psB.tile([P, T], F32, tag="y", name="uq_ps")
                for ch in range(NCH):
                    for k in range(KC):
                        nc.tensor.matmul(
                            ups[:R, c512(ch)], aq_sb[:, k, :], xTs[k][:, c512(ch)],
                            start=(k == 0), stop=(k == KC - 1),
                        )
                uq_sb = small.tile([R, T], F32R, tag="uq", name="uq_sb")
                nc.scalar.copy(uq_sb[:], ups[:R, :])
            if has_lf:
                ufs = psB.tile([P, T], F32, tag="y", name="uf_ps")
                for k in range(KC):
                    nc.tensor.matmul(
                        ufs[:R, :S], af_sb[:, k, :], fTs[k][:],
                        start=(k == 0), stop=(k == KC - 1),
                    )
                uf_sb = small.tile([R, S], F32R, tag="uf", name="uf_sb")
                nc.scalar.copy(uf_sb[:], ufs[:R, :S])

            kTs = [small.tile([P, S], F32R, tag=f"kT{m}", name=f"kT{m}") for m in range(KC)]
            for m in range(KC):
                wk_m = wcol.tile([P, KC, P], F32R, tag="wcol", name=f"wk{m}")
                nc.sync.dma_start(wk_m[:], WfkT3[:, :, m * P:(m + 1) * P])
                ps = psA.tile([P, S], F32, tag="mm", name=f"k_ps{m}")
                for k in range(KC):
                    nc.tensor.matmul(
                        ps[:], wk_m[:, k, :], fTs[k][:],
                        start=(k == 0), stop=(k == KC - 1 and not has_lf),
                    )
                if has_lf:
                    nc.tensor.matmul(
                        ps[:], bfks_sb[:, m * P:(m + 1) * P], uf_sb[:],
                        start=False, stop=True,
                    )
                if has_bfk:
                    nc.scalar.activation(
                        kTs[m][:], ps[:], mybir.ActivationFunctionType.Identity,
                        bias=bfk_sb[:, m:m + 1], scale=1.0,
                    )
                else:
                    nc.vector.tensor_copy(kTs[m][:], ps[:])

            wfv = [wrow.tile([P, C], F32R, tag="wrow", name=f"wfv{k}") for k in range(KC)]
            for k in range(KC):
                nc.sync.dma_start(wfv[k][:], WfvT3[k])
            v_aug = [
                small.tile([P, H, D + 1], F32R, tag=f"vaug{s2}", name=f"vaug{s2}")
                for s2 in range(2)
            ]
            for s2 in range(2):
                nc.sync.dma_start(v_aug[s2][:, :, D], vones[:, :])
                for ch in range(NCH):
                    ps = psA.tile([P, 512], F32, tag="mm", name=f"v_ps{s2}_{ch}")
                    nmm = KC + (1 if has_lf else 0) + (1 if has_bfv else 0)
                    i = 0
                    for k in range(KC):
                        i += 1
                        nc.tensor.matmul(
                            ps[:], fTs[k][:, s2 * P:(s2 + 1) * P],
                            wfv[k][:, c512(ch)],
                            start=(i == 1), stop=(i == nmm),
                        )
                    if has_lf:
                        i += 1
                        nc.tensor.matmul(
                            ps[:], uf_sb[:, s2 * P:(s2 + 1) * P],
                            bfvs_sb[:, c512(ch)], start=False, stop=(i == nmm),
                        )
                    if has_bfv:
                        i += 1
                        nc.tensor.matmul(
                            ps[:], ones1[:], bfv_sb[:, c512(ch)],
                            start=False, stop=(i == nmm),
                        )
                    for hh in range(8):
                        h = ch * 8 + hh
                        nc.vector.tensor_copy(
                            v_aug[s2][:, h, 0:D], ps[:, hh * D:(hh + 1) * D]
                        )

            qTs = [qpool.tile([P, T], F32R, tag="qT", name=f"qT{m}") for m in range(MT)]
            for m in range(KC):
                wq_m = wcol.tile([P, KC, P], F32R, tag="wcol", name=f"wq{m}")
                nc.sync.dma_start(wq_m[:], WqT3[:, :, m * P:(m + 1) * P])
                for ch in range(NCH):
                    ps = psA.tile([P, 512], F32, tag="mm", name=f"q_ps{m}_{ch}")
                    for k in range(KC):
                        nc.tensor.matmul(
                            ps[:], wq_m[:, k, :], xTs[k][:, c512(ch)],
                            start=(k == 0), stop=(k == KC - 1 and not has_lq),
                        )
                    if has_lq:
                        nc.tensor.matmul(
                            ps[:], bqs_sb[:, m * P:(m + 1) * P], uq_sb[:, c512(ch)],
                            start=False, stop=True,
                        )
                    if has_bq:
                        nc.scalar.activation(
                            qTs[m][:, c512(ch)], ps[:],
                            mybir.ActivationFunctionType.Identity,
                            bias=bq_sb[:, m:m + 1], scale=1.0,
                        )
                    else:
                        nc.vector.tensor_copy(qTs[m][:, c512(ch)], ps[:])

            yTr = [big.tile([P, T], F32R, tag="big", name=f"yTr{p}") for p in range(KC)]
            r_sb = small.tile([H, T], F32R, tag="rsum", name="r_sb")
            for h in range(H):
                m, off = h // 2, (h % 2) * D
                kt_h = kTs[m][off:off + D, :]
                qt_h = qTs[m][off:off + D, :]
                es = [expp.tile([P, T], F32R, tag="exp", name=f"e{h}_{s2}") for s2 in range(2)]
                for s2 in range(2):
                    for ch in range(NCH):
                        ps = psA.tile([P, 512], F32, tag="mm", name=f"s_ps{h}_{s2}_{ch}")
                        nc.tensor.matmul(
                            ps[:], kt_h[:, s2 * P:(s2 + 1) * P], qt_h[:, c512(ch)],
                            start=True, stop=True,
                        )
                        nc.scalar.activation(
                            es[s2][:, c512(ch)], ps[:],
                            mybir.ActivationFunctionType.Exp, scale=0.125,
                        )
                nc.vector.tensor_mul(es[0][:, 0:P], es[0][:, 0:P], mask_sb[:, 0:P])
                nc.vector.tensor_mul(es[1][:, 0:S], es[1][:, 0:S], mask_sb[:, P:384])
                psy = psB.tile([P, T], F32, tag="y", name=f"y_ps{h}")
                for ch in range(NCH):
                    for s2 in range(2):
                        nc.tensor.matmul(
                            psy[:D + 1, c512(ch)], v_aug[s2][:, h, :],
                            es[s2][:, c512(ch)], start=(s2 == 0), stop=(s2 == 1),
                        )
                st = stg.tile([P, T], F32R, tag="hstage", name=f"st{h}")
                if off == 0:
                    nc.vector.tensor_copy(yTr[m][0:D, :], psy[0:D, :])
                    nc.vector.tensor_copy(st[D:D + 1, :], psy[D:D + 1, :])
                else:
                    nc.vector.tensor_copy(st[0:D + 1, :], psy[0:D + 1, :])
                    nc.sync.dma_start(yTr[m][off:off + D, :], st[0:D, :])
                nc.sync.dma_start(r_sb[h:h + 1, :], st[D:D + 1, :])

            recf = small.tile([H, T], F32, tag="recf", name="recf")
            nc.vector.reciprocal(recf[:], r_sb[:])
            rec = small.tile([H, T], F32R, tag="rec", name="rec")
            nc.vector.tensor_copy(rec[:], recf[:])
            for p in range(KC):
                rb = psB.tile([P, T], F32, tag="y", name=f"rb{p}")
                for ch in range(NCH):
                    nc.tensor.matmul(
                        rb[:, c512(ch)], esel_sb[:, p * P:(p + 1) * P],
                        rec[:, c512(ch)], start=True, stop=True,
                    )
                nc.vector.tensor_mul(yTr[p][:], yTr[p][:], rb[:])

            if has_lp:
                upsd = psB.tile([P, T], F32, tag="y", name="up_ps")
                for ch in range(NCH):
                    for k in range(KC):
                        nc.tensor.matmul(
                            upsd[:R, c512(ch)], ap_sb[:, k, :], yTr[k][:, c512(ch)],
                            start=(k == 0), stop=(k == KC - 1),
                        )
                up_sb = small.tile([R, T], F32R, tag="up", name="up_sb")
                nc.scalar.copy(up_sb[:], upsd[:R, :])
            wp = [wrow.tile([P, C], F32R, tag="wrow", name=f"wp{k}") for k in range(KC)]
            for k in range(KC):
                nc.sync.dma_start(wp[k][:], WpT3[k])
            for m in range(MT):
                for ch in range(NCH):
                    ps = psA.tile([P, 512], F32, tag="mm", name=f"o_ps{m}_{ch}")
                    nmm = KC + (1 if has_lp else 0) + (1 if has_bp else 0)
                    i = 0
                    for k in range(KC):
                        i += 1
                        nc.tensor.matmul(
                            ps[:], yTr[k][:, m * P:(m + 1) * P], wp[k][:, c512(ch)],
                            start=(i == 1), stop=(i == nmm),
                        )
                    if has_lp:
                        i += 1
                        nc.tensor.matmul(
                            ps[:], up_sb[:, m * P:(m + 1) * P], bps_sb[:, c512(ch)],
                            start=False, stop=(i == nmm),
                        )
                    if has_bp:
                        i += 1
                        nc.tensor.matmul(
                            ps[:], ones1[:], bp_sb[:, c512(ch)],
                            start=False, stop=(i == nmm),
                        )
                    ost = wcol.tile([P, 512], F32, tag="ostage", name=f"ost{m}_{ch}")
                    nc.scalar.copy(ost[:], ps[:])
                    nc.sync.dma_start(out[m * P:(m + 1) * P, c512(ch)], ost[:])

    nc.finalize()
    return nc


def _host_prep_slow(x, feature, Wq, bq, Aq, Bq, Wf, bf, Af, Bf, Wp, bp, Ap, Bp, flags):
    f32 = np.float32
    shared = {
        "WqT": np.ascontiguousarray(np.asarray(Wq, f32).T),
        "WfkT": np.ascontiguousarray(np.asarray(Wf[:C], f32).T),
        "WfvT": np.ascontiguousarray(np.asarray(Wf[C:], f32).T),
        "WpT": np.ascontiguousarray(np.asarray(Wp, f32).T),
    }
    i = np.arange(P)[:, None]
    j = np.arange(384)[None, :]
    m0 = (j[:, :P] >= i).astype(f32)
    m1 = ((j[:, P:384] - P) >= (P + i)).astype(f32)
    shared["mask"] = np.ascontiguousarray(np.concatenate([m0, m1], axis=1))
    shared["vones"] = np.ones((P, H), f32)
    hsel = np.arange(H)[:, None]
    col = np.arange(C)[None, :]
    shared["Esel"] = np.ascontiguousarray((hsel == col // D).astype(f32))
    has_lq, has_lf, has_lp, has_bq, has_bfk, has_bfv, has_bp = flags
    if has_lq:
        shared["AqT"] = np.ascontiguousarray(np.asarray(Aq, f32).T)
        shared["BqTs"] = np.ascontiguousarray(np.asarray(Bq, f32).T * SCALING)
    if has_lf:
        shared["AfT"] = np.ascontiguousarray(np.asarray(Af, f32).T)
        shared["BfkTs"] = np.ascontiguousarray(np.asarray(Bf[:C], f32).T * SCALING)
        shared["BfvTs"] = np.ascontiguousarray(np.asarray(Bf[C:], f32).T * SCALING)
    if has_lp:
        shared["ApT"] = np.ascontiguousarray(np.asarray(Ap, f32).T)
        shared["BpTs"] = np.ascontiguousarray(np.asarray(Bp, f32).T * SCALING)
    if has_bq:
        shared["bq_pp"] = np.ascontiguousarray(np.asarray(bq, f32).reshape(KC, P).T)
    if has_bfk:
        shared["bfk_pp"] = np.ascontiguousarray(np.asarray(bf[:C], f32).reshape(KC, P).T)
    if has_bfv:
        shared["bfv_row"] = np.ascontiguousarray(np.asarray(bf[C:], f32).reshape(1, C))
    if has_bp:
        shared["bp_row"] = np.ascontiguousarray(np.asarray(bp, f32).reshape(1, C))

    in_maps = []
    for b in range(B):
        m = dict(shared)
        m["xT"] = np.ascontiguousarray(np.asarray(x[b], f32).T)
        m["fT"] = np.ascontiguousarray(np.asarray(feature[b], f32).T)
        in_maps.append(m)
    return in_maps


# --------------------------------------------------------------------------
# dispatch
# --------------------------------------------------------------------------

def _run(inputs, trace=False, **spmd_kwargs):
    x, feature = inputs["x"], inputs["feature"]
    Wq, bq, Aq, Bq = inputs["Wq"], inputs["bq"], inputs["Aq"], inputs["Bq"]
    Wf, bf_, Af, Bf = inputs["Wf"], inputs["bf"], inputs["Af"], inputs["Bf"]
    Wp, bp, Ap, Bp = inputs["Wp"], inputs["bp"], inputs["Ap"], inputs["Bp"]
    flags = (
        bool(np.any(Bq)), bool(np.any(Bf)), bool(np.any(Bp)),
        bool(np.any(bq)), bool(np.any(bf_[:C])), bool(np.any(bf_[C:])),
        bool(np.any(bp)),
    )
    if any(flags):
        key = ("slow", flags)
        nc = _nc_cache.get(key)
        if nc is None:
            nc = _build_slow(flags)
            _nc_cache[key] = nc
        in_maps = _host_prep_slow(
            x, feature, Wq, bq, Aq, Bq, Wf, bf_, Af, Bf, Wp, bp, Ap, Bp, flags
        )
    else:
        key = "fast"
        nc = _nc_cache.get(key)
        if nc is None:
            nc = _build_fast()
            _nc_cache[key] = nc
        in_maps = _host_prep_fast(x, feature, Wq, Wf, Wp)
    res = run_bass_kernel_spmd(
        nc, in_maps, core_ids=list(range(B)), trace=trace, **spmd_kwargs
    )
    out = np.stack([res.results[b]["out"] for b in range(B)], axis=0)
    return out, res


def kernel(**inputs):
    out, _ = _run(inputs, trace=False)
    return out
